# revision 56
# baseline (speedup 1.0000x reference)
"""Trainium2 Bass kernel for a dense transformer decoder block.

Reference computation (B=4, S=2048, D=768, H=12, DK=64, DF=3072):
    q,k,v = x@wq+bq, x@wk+bk, x@wv+bv          (per-head split, DK=64)
    attn  = softmax(mask(q k^T / 8))
    ctx   = attn @ v
    h     = LN(ctx@wo + bo + x; g1, be1)
    out   = LN(gelu_exact(h@w1 + b1)@w2 + b2 + h; g2, be2)

Sharding: pure data parallel, zero collectives. 8 cores = 4 batch elements
x 2 query groups of 1024 rows. Queries are paired so each core's two
512-row query blocks need key extents {<=8, <=16} key-blocks of 128
(block-causal skip); the exact mask is applied as data.
Core 2b+0: query rows [0:512) u [1536:2048) of batch b.
Core 2b+1: query rows [512:1536) of batch b.
Every core runs the identical SPMD program; per-core behavior differs only
through input data (sliced/transposed/cast on the host).

Schedule: attention is ACT-(exp)-bound, so independent PE work is woven
between attention iterations to keep the tensor engine dense (and its HAM
clock warm): the sb2/sb3 K,V projections run under qb0 attention, and the
qb0 out-projection + LN1 + h-transposes run under qb1 attention.
"""

from contextlib import ExitStack

import numpy as np
import ml_dtypes

import concourse.bass as bass
import concourse.tile as tile
from concourse import bacc, mybir
from concourse.bass_utils import run_bass_kernel_spmd
from concourse.masks import make_identity

F32 = mybir.dt.float32
I32 = mybir.dt.int32
BF16 = mybir.dt.bfloat16
F8E4 = mybir.dt.float8e4
AF = mybir.ActivationFunctionType
OP = mybir.AluOpType
BF = ml_dtypes.bfloat16

B, S, D, H, DK = 4, 2048, 768, 12, 64
DF = 4 * D
EPS = 1e-5
P = 128
SQ = 1024            # query rows per core
HP = H // 2          # 6 head pairs
KB = S // P          # 16 key blocks
QB = 2               # query blocks of 512 per core
QBS = 512
EXT = [8, 16]        # key-block extent per query block (block-causal skip)
DC = D // P          # 6 chunks of the model dim
DFC = DF // P        # 24 chunks of the FFN dim
QC = SQ // P         # 8 query chunks of 128
NH = 2               # 384-wide halves of D for PSUM-friendly matmul N
NHW = D // NH        # 384
SB = S // QBS        # 4 key column slabs

N_CORES = 8


def emit(ctx: ExitStack, tc: tile.TileContext, io: dict):
    nc = tc.nc

    xT, xqT, xres, maskT = io["xT"], io["xqT"], io["xres"], io["maskT"]
    wq, wk, wv, wo, w1, w2 = io["wq"], io["wk"], io["wv"], io["wo"], io["w1"], io["w2"]
    out = io["out"]

    # ---- constants ----------------------------------------------------
    const = ctx.enter_context(tc.tile_pool(name="const", bufs=1))
    ident = const.tile([P, P], BF16)
    make_identity(nc, ident)
    eps_t = const.tile([P, 1], F32)
    nc.vector.memset(eps_t, EPS)

    # PE warmup: the first input DMA bytes land ~9us in and the first
    # projection matmul issues ~22us in; without sustained PE activity the
    # HAM clock gate holds the array at 1.2GHz for the first ~3.4us of real
    # work. Junk matmuls on the on-chip identity span the DMA wait so the
    # projections start at 2.4GHz.
    warm_ps = tc.alloc_tile_pool(name="warm_ps", bufs=1, space="PSUM")
    wps = warm_ps.tile([64, 64], F32)
    for _ in range(120):
        nc.tensor.matmul(wps, lhsT=ident[:, 0:64], rhs=ident[:, 0:64],
                         start=True, stop=True)
    warm_ps.release()

    bqp = const.tile([P, HP], F32)
    nc.gpsimd.dma_start(out=bqp, in_=io["bqp"])
    bkp = const.tile([P, HP], F32)
    nc.gpsimd.dma_start(out=bkp, in_=io["bkp"])
    b1p = const.tile([P, DFC], F32)

    def brow_alloc(name):
        # [1, D] dram tensor broadcast-DMA'd across 128 partitions; tiles
        # allocated here, transfers issued after the projection inputs so
        # the first matmuls aren't queued behind 1.9MB of LN constants
        return const.tile([P, D], F32, tag=name, name=name)

    g1b, be1b, g2b, be2b, b2b = map(
        brow_alloc, ["g1r", "be1r", "g2r", "be2r", "b2r"])
    _brow_tiles = {"g1r": g1b, "be1r": be1b, "g2r": g2b, "be2r": be2b,
                   "b2r": b2b}

    def issue_ln_const_dmas():
        nc.gpsimd.dma_start(out=b1p, in_=io["b1p"])
        for name, t in _brow_tiles.items():
            a = io[name]
            src = bass.AP(tensor=a.tensor, offset=a.offset,
                          ap=[[0, P]] + list(a.ap[1:]))
            nc.gpsimd.dma_start(out=t, in_=src)

    # ---- FFN-phase tensors: left stack, below attn_in so release order
    # stays LIFO (h/hT are written during the attention epilogue fillers)
    ffn = tc.alloc_tile_pool(name="ffn", bufs=1)
    h_sb = ffn.tile([P, QC, D], BF16)     # LN1 out (residual + FFN rhs)
    hT = ffn.tile([P, DC, SQ], BF16)
    ln_wk = tc.alloc_tile_pool(name="ln_wk", bufs=1)

    # ---- attention inputs (live through attention) --------------------
    attn_in = tc.alloc_tile_pool(name="attn_in", bufs=1)
    KT = attn_in.tile([P, HP, S], BF16)            # K^T, head pairs on partitions
    Vaug = attn_in.tile([P, KB, H, DK + 1], BF16)  # V + ones column per head
    QT = attn_in.tile([P, HP, SQ], BF16)
    # only the mask quadrants that aren't structurally skipped:
    # qb0 masks key blocks 0..7, qb1 masks key blocks 8..15
    mT1 = attn_in.tile([P, 8, QBS], BF16)
    mr = maskT.rearrange("(kb p) q -> p kb q", p=P)
    nc.vector.memset(Vaug[:, :, :, DK : DK + 1], 1.0)

    # ---- post-attention inputs (right-side stack, phase-scoped) -------
    mid_ctx = tc.alloc_tile_pool(name="mid_ctx", bufs=1, side="right")
    ctxT = mid_ctx.tile([P, DC, SQ], BF16)

    kv_in = tc.alloc_tile_pool(name="kv_in", bufs=1, side="right")
    wk_sb = kv_in.tile([P, DC, D], BF16)
    wv_sb = kv_in.tile([P, DC, D], BF16)
    xT23 = kv_in.tile([P, DC, S // 2], BF16)
    xt01p = tc.alloc_tile_pool(name="xt01p", bufs=1, side="right")
    xT01 = xt01p.tile([P, DC, S // 2], BF16)
    xTr = xT.rearrange("(c p) s -> p c s", p=P)

    def xT_at(sb):
        t = xT01 if sb < 2 else xT23
        return t, (sb % 2) * QBS

    q_in = tc.alloc_tile_pool(name="q_in", bufs=1, side="right")
    wq_sb = q_in.tile([P, DC, D], BF16)
    xqT_sb = q_in.tile([P, DC, SQ], BF16)
    # wq lands as 6 per-head-pair column slices: q_unit(hp0) only needs
    # slice 0 (200KB), so the first projection starts ~9us earlier than
    # waiting for the whole 1.2MB transfer
    wqr = wq.rearrange("(c p) n -> p c n", p=P)
    for hp in range(HP):
        nc.sync.dma_start(out=wq_sb[:, :, hp * P : (hp + 1) * P],
                          in_=wqr[:, :, hp * P : (hp + 1) * P])
    xqr = xqT.rearrange("(c p) s -> p c s", p=P)
    for sb in range(2):
        nc.scalar.dma_start(out=xqT_sb[:, :, sb * QBS : (sb + 1) * QBS],
                            in_=xqr[:, :, sb * QBS : (sb + 1) * QBS])
    for sb in range(SB):
        dst = xT01 if sb < 2 else xT23
        nc.sync.dma_start(out=dst[:, :, (sb % 2) * QBS : (sb % 2 + 1) * QBS],
                          in_=xTr[:, :, sb * QBS : (sb + 1) * QBS])
    # wk on the gpsimd queue, wv on scalar: spreads the projection-input
    # load across three DGE queues instead of two
    for c in range(DC):
        nc.gpsimd.dma_start(out=wk_sb[:, c, :],
                            in_=wk.rearrange("(c p) n -> p c n", p=P)[:, c, :])
    for c in range(DC):
        nc.gpsimd.dma_start(out=wv_sb[:, c, :],
                            in_=wv.rearrange("(c p) n -> p c n", p=P)[:, c, :])
    # mask + LN constants: not needed until well after the projections,
    # so these transfers queue behind the weight/x loads
    issue_ln_const_dmas()
    nc.gpsimd.dma_start(out=mT1, in_=mr[:, 8:16, QBS : 2 * QBS])

    def layer_norm(wk_pool, src, gb, bb, dst, act_stats=False):
        # LN over the free dim (768) of fp32 src [128, 768]; dst may be the
        # same tile or a bf16 target
        mv = wk_pool.tile([P, 2], F32, tag="mv", bufs=3)
        if act_stats:
            # stats on ACT (idle in the FFN phase): two accumulating passes
            # give sum(x) and sum(x^2); relieves the DVE-bound LN2 drain
            # that follows the last FFN matmuls
            scr = wk_pool.tile([P, D], F32, tag="lnscr", bufs=2)
            s1 = wk_pool.tile([P, 1], F32, tag="s1", bufs=3)
            s2 = wk_pool.tile([P, 1], F32, tag="s2", bufs=3)
            nc.scalar.activation(out=scr, in_=src, func=AF.Square,
                                 accum_out=s2)
            nc.scalar.activation(out=scr, in_=src, func=AF.Identity,
                                 accum_out=s1)
            nc.vector.tensor_scalar_mul(out=mv[:, 0:1], in0=s1,
                                        scalar1=1.0 / D)
            musq = wk_pool.tile([P, 1], F32, tag="musq", bufs=3)
            nc.vector.tensor_tensor(out=musq, in0=mv[:, 0:1],
                                    in1=mv[:, 0:1], op=OP.mult)
            nc.vector.tensor_scalar(out=mv[:, 1:2], in0=s2,
                                    scalar1=1.0 / D, scalar2=musq,
                                    op0=OP.mult, op1=OP.subtract)
        else:
            stats = wk_pool.tile([P, 2, 6], F32, tag="stats", bufs=3)
            for j in range(2):
                nc.vector.bn_stats(out=stats[:, j, :],
                                   in_=src[:, j * 384 : (j + 1) * 384])
            nc.vector.bn_aggr(out=mv, in_=stats)
        # rstd = rsqrt(var+eps) entirely on DVE (Quake seed + 3 Newton
        # steps, ~1e-7 rel err): an ACT Sqrt here would force a ~2.7us
        # activation-table swap against the attention exps / FFN gelus
        vh = wk_pool.tile([P, 1], F32, tag="vh", bufs=3)
        nc.vector.tensor_scalar(out=vh, in0=mv[:, 1:2], scalar1=EPS,
                                scalar2=-0.5, op0=OP.add, op1=OP.mult)
        seed = wk_pool.tile([P, 1], I32, tag="seed", bufs=3)
        nc.vector.tensor_scalar(out=seed, in0=mv[:, 1:2].bitcast(I32),
                                scalar1=1, scalar2=None,
                                op0=OP.logical_shift_right)
        nc.vector.tensor_scalar(out=seed, in0=seed, scalar1=-1,
                                scalar2=0x5F3759DF, op0=OP.mult, op1=OP.add)
        rstd = wk_pool.tile([P, 1], F32, tag="rstd", bufs=3)
        y = seed.bitcast(F32)
        for _ in range(3):
            w = wk_pool.tile([P, 1], F32, tag="nw", bufs=3)
            nc.vector.scalar_tensor_tensor(out=w, in0=y, scalar=y,
                                           in1=vh, op0=OP.mult, op1=OP.mult)
            nc.vector.tensor_scalar(out=w, in0=w, scalar1=1.5, scalar2=None,
                                    op0=OP.add)
            nc.vector.tensor_tensor(out=rstd, in0=y, in1=w, op=OP.mult)
            y = rstd
        nc.vector.tensor_scalar_sub(out=src, in0=src, scalar1=mv[:, 0:1])
        nc.vector.scalar_tensor_tensor(out=src, in0=src, scalar=rstd,
                                       in1=gb, op0=OP.mult, op1=OP.mult)
        nc.vector.tensor_tensor(out=dst, in0=src, in1=bb, op=OP.add)

    proj_ps = tc.alloc_tile_pool(name="proj_ps", bufs=2, space="PSUM", side="right")
    with tc.tile_pool(name="sc_ps", bufs=2, space="PSUM") as sc_ps, \
         tc.tile_pool(name="cx_ps", bufs=1, space="PSUM") as cx_ps:
        # at_sb/nm_sb/mT0 are allocated only once the q-projection inputs are
        # released -- their SBUF footprints must not overlap
        pools = {}

        # ---------- projection work units ----------
        def q_unit(hp, sb):
            ps = proj_ps.tile([P, QBS], F32, tag="proj")
            for c in range(DC):
                nc.tensor.matmul(
                    ps, lhsT=wq_sb[:, c, hp * P : (hp + 1) * P],
                    rhs=xqT_sb[:, c, sb * QBS : (sb + 1) * QBS],
                    start=(c == 0), stop=(c == DC - 1),
                )
            # bias on DVE: the ACT FIFO is clogged with DMA-descriptor and
            # semaphore instructions at startup, which left the PE stalled
            # ~8us on the proj_ps ring waiting for the first bias reads
            nc.vector.tensor_scalar_add(
                out=QT[:, hp, sb * QBS : (sb + 1) * QBS], in0=ps,
                scalar1=bqp[:, hp : hp + 1],
            )

        def k_unit(hp, sb, on_act=True):
            xt, off = xT_at(sb)
            ps = proj_ps.tile([P, QBS], F32, tag="proj")
            for c in range(DC):
                nc.tensor.matmul(
                    ps, lhsT=wk_sb[:, c, hp * P : (hp + 1) * P],
                    rhs=xt[:, c, off : off + QBS],
                    start=(c == 0), stop=(c == DC - 1),
                )
            if on_act:
                nc.scalar.activation(
                    out=KT[:, hp, sb * QBS : (sb + 1) * QBS], in_=ps,
                    func=AF.Identity, bias=bkp[:, hp : hp + 1],
                )
            else:
                # inside the attention interleave ACT is the bottleneck chain
                nc.vector.tensor_scalar_add(
                    out=KT[:, hp, sb * QBS : (sb + 1) * QBS], in0=ps,
                    scalar1=bkp[:, hp : hp + 1],
                )

        def v_unit(kb, nh, on_act=True):
            xt, off = xT_at(kb // (QBS // P))
            kb_off = off // P + kb % (QBS // P)
            ps = proj_ps.tile([P, QBS], F32, tag="proj")
            psv = ps[:, 0:NHW]
            for c in range(DC):
                nc.tensor.matmul(
                    psv, lhsT=xt[:, c, kb_off * P : (kb_off + 1) * P],
                    rhs=wv_sb[:, c, nh * NHW : (nh + 1) * NHW],
                    start=(c == 0), stop=(c == DC - 1),
                )
            if on_act:
                nc.scalar.activation(
                    out=Vaug[:, kb, nh * 6 : (nh + 1) * 6, 0:DK],
                    in_=psv.rearrange("p (h d) -> p h d", d=DK),
                    func=AF.Copy,
                )
            else:
                nc.vector.tensor_copy(
                    out=Vaug[:, kb, nh * 6 : (nh + 1) * 6, 0:DK],
                    in_=psv.rearrange("p (h d) -> p h d", d=DK),
                )

        def kv_slab(sb, on_act=True):
            for hp in range(HP):
                k_unit(hp, sb, on_act)
            for j in range(QBS // P):
                for nh in range(NH):
                    v_unit(sb * (QBS // P) + j, nh, on_act)

        # ---------- attention iteration ----------
        pending = []

        def make_norm(cxs_e, cxs_o, den2, hp, qs):
            def go():
                # one reciprocal serves both heads: its cost scales with the
                # free size, not the partition count
                rec2 = pools['nm_sb'].tile([DK + 1, QBS], F32, tag="rec2", bufs=1)
                # softmax denominators are well-conditioned (>=1, <~4e3):
                # the ~5x faster 18-bit approx is far inside tolerance
                nc.vector.reciprocal_approx_fast(rec2, den2)
                # partition_broadcast replicates the tile's physical partition
                # 0, so the head-odd reciprocal must move to its own base-0
                # tile first
                rec_o = pools['nm_sb'].tile([1, QBS], F32, tag="rec_o", bufs=1)
                nc.vector.tensor_copy(out=rec_o, in_=rec2[DK : DK + 1, :])
                for i, (cxs, pb) in enumerate(((cxs_e, 0), (cxs_o, DK))):
                    src_r = rec2[0:1, :] if i == 0 else rec_o[0:1, :]
                    den_b = pools['nm_sb'].tile([DK, QBS], F32, tag="den_b", bufs=1)
                    nc.gpsimd.partition_broadcast(den_b, src_r)
                    nc.vector.tensor_tensor(
                        out=ctxT[pb : pb + DK, hp, qs], in0=cxs[0:DK, :],
                        in1=den_b, op=OP.mult,
                    )
            return go

        def attn_iter(hp, qb, fill=None):
            ext = EXT[qb]
            qs = slice(qb * QBS, (qb + 1) * QBS)
            cx_e = cx_ps.tile([DK + 1, QBS], F32, tag="cx_e")
            cx_o = cx_ps.tile([DK + 1, QBS], F32, tag="cx_o")
            def front(gb):
                # scores + exp (+ mask) for key-block pair gb
                pt = pools['at_sb'].tile([P, 2, 2, QBS], BF16, tag="pt")
                for gi in range(2):
                    g = gb + gi
                    ks = slice(g * P, (g + 1) * P)
                    sc = sc_ps.tile([P, 2, QBS], F32, tag="sc")
                    # the two heads of a pair hit disjoint PE row groups and
                    # run concurrently in the array
                    nc.tensor.matmul(sc[:, 0, :], lhsT=KT[0:DK, hp, ks],
                                     rhs=QT[0:DK, hp, qs], start=True, stop=True)
                    nc.tensor.matmul(sc[:, 1, :], lhsT=KT[DK:P, hp, ks],
                                     rhs=QT[DK:P, hp, qs], start=True, stop=True)
                    nc.scalar.activation(out=pt[:, :, gi, :], in_=sc,
                                         func=AF.Exp, scale=1.0 / 8.0)
                # qb0 masks kb 0..7 via mT0; qb1 masks only kb 8..15 (below
                # the diagonal for every core) via mT1
                if qb == 0 or gb >= 8:
                    mq = pools['mT0'][:, gb : gb + 2, :] if qb == 0 else \
                        mT1[:, gb - 8 : gb - 6, :]
                    # one multiply for both heads: the mask AP broadcasts
                    # over the head dim with a 0-stride
                    mqb = bass.AP(tensor=mq.tensor, offset=mq.offset,
                                  ap=[mq.ap[0], [0, 2]] + list(mq.ap[1:]))
                    nc.vector.tensor_tensor(out=pt, in0=pt, in1=mqb,
                                            op=OP.mult)
                return pt

            def back(gb, pt):
                for gi in range(2):
                    g = gb + gi
                    nc.tensor.matmul(cx_e, lhsT=Vaug[:, g, 2 * hp, :],
                                     rhs=pt[:, 0, gi, :],
                                     start=(g == 0), stop=(g == ext - 1))
                    nc.tensor.matmul(cx_o, lhsT=Vaug[:, g, 2 * hp + 1, :],
                                     rhs=pt[:, 1, gi, :],
                                     start=(g == 0), stop=(g == ext - 1))
                if gb == 2 and pending:
                    pending.pop()()
                if fill and (qb == 0 or gb % 4 == 2):
                    fill.pop(0)()

            # one-pair software pipeline: pair t+1's score matmuls issue
            # BEFORE pair t's ctx matmuls/fillers, so the exp chain on ACT
            # (the qb1 bottleneck) never starves behind PE queue order --
            # S(t,a) S(t,b) C(t-1) takes ~1.7us, just past exp(t,a)'s 1.57us
            prev = None
            for gb in range(0, ext, 2):
                pt = front(gb)
                if prev is not None:
                    back(*prev)
                prev = (gb, pt)
            back(*prev)
            # stage ctx to SBUF immediately: frees the PSUM bank within one
            # DVE copy so the cx pool gets away with a single buffer
            cxs_e = pools['nm_sb'].tile([DK + 1, QBS], F32, tag="cxs_e")
            nc.vector.tensor_copy(out=cxs_e, in_=cx_e)
            cxs_o = pools['nm_sb'].tile([DK + 1, QBS], F32, tag="cxs_o")
            nc.vector.tensor_copy(out=cxs_o, in_=cx_o)
            den2 = pools['nm_sb'].tile([DK + 1, QBS], F32, tag="den2")
            nc.vector.memset(den2, 1.0)
            nc.vector.tensor_copy(out=den2[0:1, :], in_=cx_e[DK : DK + 1, :])
            nc.vector.tensor_copy(out=den2[DK : DK + 1, :], in_=cx_o[DK : DK + 1, :])
            pending.append(make_norm(cxs_e, cxs_o, den2, hp, qs))

        # ---------- schedule: projections + qb0 attention ----------
        for hp in range(HP):
            q_unit(hp, 0)
        kv_slab(0, on_act=False)
        kv_slab(1, on_act=False)
        for hp in range(HP):
            q_unit(hp, 1)
        q_in.release()
        xt01p.release()
        pools['at_sb'] = tc.alloc_tile_pool(name="at_sb", bufs=3)
        pools['nm_sb'] = tc.alloc_tile_pool(name="nm_sb", bufs=2)
        mT0p = tc.alloc_tile_pool(name="mT0p", bufs=1)
        pools['mT0'] = mT0p.tile([P, 8, QBS], BF16, name="mT0", tag="mT0")
        nc.gpsimd.dma_start(out=pools['mT0'], in_=mr[:, 0:8, 0:QBS])
        # on_act=False: these run woven into qb0 attention where ACT is the
        # exp-bound critical chain -- their epilogues go to DVE instead
        kv_fill = [(lambda hp=hp, sb=sb: k_unit(hp, sb, on_act=False))
                   for sb in (2, 3) for hp in range(HP)] + \
                  [(lambda kb=kb, nh=nh: v_unit(kb, nh, on_act=False))
                   for kb in range(8, KB) for nh in range(NH)]
        for hp in range(HP):
            attn_iter(hp, 0, kv_fill)
        for fn in kv_fill:
            fn()
        kv_fill.clear()
        kv_in.release()
        proj_ps.release()
        mT0p.release()

        # ---------- qb1 attention with qb0 epilogue woven in ----------
        mid_ow = tc.alloc_tile_pool(name="mid_ow", bufs=1, side="right")
        xres_sb = mid_ow.tile([P, QC, D], F32)
        nc.gpsimd.dma_start(out=xres_sb,
                            in_=xres.rearrange("(c p) n -> p c n", p=P))
        wo_sb = mid_ow.tile([P, DC, D], BF16)
        nc.gpsimd.dma_start(out=wo_sb, in_=wo.rearrange("(c p) n -> p c n", p=P))
        op_ps = tc.alloc_tile_pool(name="op_ps", bufs=1, space="PSUM", side="right")
        tp_ps = tc.alloc_tile_pool(name="tp_ps", bufs=1, space="PSUM", side="right")

        hpre_map = {}

        def op_half(qc, nh):
            def go():
                if qc not in hpre_map:
                    hpre_map[qc] = ln_wk.tile([P, D], F32, tag="hpre",
                                              bufs=3, name=f"hpre_{qc}")
                hpre = hpre_map[qc]
                ps = op_ps.tile([P, NHW], F32, tag="op")
                for c in range(DC):
                    nc.tensor.matmul(
                        ps, lhsT=ctxT[:, c, qc * P : (qc + 1) * P],
                        rhs=wo_sb[:, c, nh * NHW : (nh + 1) * NHW],
                        start=(c == 0), stop=(c == DC - 1),
                    )
                nc.vector.scalar_tensor_tensor(
                    out=hpre[:, nh * NHW : (nh + 1) * NHW], in0=ps,
                    scalar=1.0, in1=xres_sb[:, qc, nh * NHW : (nh + 1) * NHW],
                    op0=OP.mult, op1=OP.add,
                )
            return go

        def ln_unit(qc):
            def go():
                layer_norm(ln_wk, hpre_map.pop(qc), g1b, be1b, h_sb[:, qc, :])
            return go

        def transp_half(qc, lo):
            def go():
                for c in range(lo, lo + DC // 2):
                    tp = tp_ps.tile([P, P], BF16, tag="tp")
                    nc.tensor.transpose(tp, h_sb[:, qc, c * P : (c + 1) * P],
                                        ident)
                    nc.scalar.activation(out=hT[:, c, qc * P : (qc + 1) * P],
                                         in_=tp, func=AF.Copy)
            return go

        def outproj_unit(qc):
            def go():
                op_half(qc, 0)()
                op_half(qc, 1)()
                ln_unit(qc)()
            return go

        def transp_unit(qc):
            def go():
                for c in range(DC):
                    tp = tp_ps.tile([P, P], BF16, tag="tp")
                    nc.tensor.transpose(tp, h_sb[:, qc, c * P : (c + 1) * P],
                                        ident)
                    nc.scalar.activation(out=hT[:, c, qc * P : (qc + 1) * P],
                                         in_=tp, func=AF.Copy)
            return go

        fillers = []
        for qc in range(4):
            fillers += [op_half(qc, 0), op_half(qc, 1), ln_unit(qc)]
        for qc in range(4):
            fillers += [transp_half(qc, 0), transp_half(qc, DC // 2)]
        for hp in range(HP):
            attn_iter(hp, 1, fillers)
        for fn in pending:
            fn()
        pending.clear()
        for fn in fillers:
            fn()
        pools['nm_sb'].release()
        pools['at_sb'].release()

    attn_in.release()

    # prefetch the 9.4MB w1/w2 load so it overlaps the qc4-7 epilogue
    # instead of stalling the FFN phase start
    w12_in = tc.alloc_tile_pool(name="w12_in", bufs=1)
    w1_sb = w12_in.tile([P, DC, DF], BF16)
    w1r = w1.rearrange("(c p) n -> p c n", p=P)
    nc.sync.dma_start(out=w1_sb[:, 0:3, :], in_=w1r[:, 0:3, :])
    nc.scalar.dma_start(out=w1_sb[:, 3:6, :], in_=w1r[:, 3:6, :])
    w2_sb = w12_in.tile([P, DFC, D], F8E4)
    nc.gpsimd.dma_start(out=w2_sb, in_=w2.rearrange("(c p) n -> p c n", p=P))

    # ====== FFN: f1^T = gelu(w1^T h^T + b1); out = LN2(f1g^T w2 + h) ====
    with tc.tile_pool(name="f1_ps", bufs=3, space="PSUM") as f1_ps, \
         tc.tile_pool(name="f2_ps", bufs=3, space="PSUM") as f2_ps, \
         tc.tile_pool(name="f1g_sb", bufs=2) as f1g_sb, \
         tc.tile_pool(name="out_sb", bufs=3) as out_sb:

        def ffn1(qb, fill=None):
            qs = slice(qb * QBS, (qb + 1) * QBS)
            # fp8: FC2 runs as DoubleRow (2 k-tiles/pass, ~1.4x); gelu
            # outputs |g|<~8 and w2~0.02 are far inside e4m3 range, and the
            # FC2-only quantization error (~1.4e-2) fits the 2e-2 tolerance
            f1g = f1g_sb.tile([P, DFC, QBS], F8E4, tag="f1g",
                              name=f"f1g_{qb}")
            for f in range(DFC):
                ps = f1_ps.tile([P, QBS], F32, tag="f1")
                for c in range(DC):
                    nc.tensor.matmul(
                        ps, lhsT=w1_sb[:, c, f * P : (f + 1) * P],
                        rhs=hT[:, c, qs], start=(c == 0), stop=(c == DC - 1),
                    )
                nc.scalar.activation(out=f1g[:, f, :], in_=ps, func=AF.Gelu,
                                     bias=b1p[:, f : f + 1])
                if fill:
                    fill.pop(0)()
            return f1g

        def ffn2(qb, f1g):
            for sq in range(QBS // P):
                qc = qb * (QBS // P) + sq
                ot = out_sb.tile([P, D], F32, tag="ot")
                for nh in range(NH):
                    ps = f2_ps.tile([P, NHW], F32, tag="f2")
                    for fp in range(DFC // 2):
                        nc.tensor.matmul(
                            ps,
                            lhsT=f1g[:, 2 * fp : 2 * fp + 2,
                                     sq * P : (sq + 1) * P],
                            rhs=w2_sb[:, 2 * fp : 2 * fp + 2,
                                      nh * NHW : (nh + 1) * NHW],
                            start=(fp == 0), stop=(fp == DFC // 2 - 1),
                            perf_mode=mybir.MatmulPerfMode.DoubleRow,
                        )
                    nc.vector.scalar_tensor_tensor(
                        out=ot[:, nh * NHW : (nh + 1) * NHW], in0=ps,
                        scalar=1.0,
                        in1=h_sb[:, qc, nh * NHW : (nh + 1) * NHW],
                        op0=OP.mult, op1=OP.add,
                    )
                nc.vector.tensor_tensor(out=ot, in0=ot, in1=b2b, op=OP.add)
                layer_norm(ln_wk, ot, g2b, be2b, ot, act_stats=True)
                nc.sync.dma_start(out=out[qc * P : (qc + 1) * P, :], in_=ot)

        # ---- qc4-7 out-projection epilogue woven INTO FFN1(qb0): FFN1 qb0
        # only needs hT qc0-3 (built during the qb1 fillers). Each op_half
        # rides between two dense f-units so its bufs=1 PSUM ring drains
        # behind PE work instead of serializing against the DVE adds
        epi = []
        for qc in range(4, QC):
            epi += [op_half(qc, 0), op_half(qc, 1), ln_unit(qc)]
        f1g0 = ffn1(0, epi)
        for qc in range(4, QC):
            transp_unit(qc)()
        ffn2(0, f1g0)
        ffn2(1, ffn1(1))

    tp_ps.release()
    op_ps.release()
    mid_ow.release()
    mid_ctx.release()

    w12_in.release()
    ln_wk.release()
    ffn.release()


def build_program():
    nc = bacc.Bacc("TRN2", target_bir_lowering=False, debug=False,
                   enable_asserts=False, num_devices=N_CORES)
    io = {}

    def din(name, shape, dt):
        io[name] = nc.dram_tensor(name, list(shape), dt, kind="ExternalInput").ap()

    din("xT", (D, S), BF16)
    din("xqT", (D, SQ), BF16)
    din("xres", (SQ, D), F32)
    din("maskT", (S, SQ), BF16)
    din("wq", (D, D), BF16)
    din("wk", (D, D), BF16)
    din("wv", (D, D), BF16)
    din("wo", (D, D), BF16)
    din("w1", (D, DF), BF16)
    din("w2", (DF, D), F8E4)
    din("bqp", (P, HP), F32)
    din("bkp", (P, HP), F32)
    din("b1p", (P, DFC), F32)
    for n in ["g1r", "be1r", "g2r", "be2r", "b2r"]:
        din(n, (1, D), F32)
    io["out"] = nc.dram_tensor("out", [SQ, D], F32, kind="ExternalOutput").ap()

    with tile.TileContext(nc) as tc:
        with ExitStack() as ctx:
            emit(ctx, tc, io)
    nc.compile()
    return nc


_NC = None


def _get_program():
    global _NC
    if _NC is None:
        _NC = build_program()
    return _NC


def _qrows(half):
    if half == 0:
        return np.concatenate([np.arange(0, 512), np.arange(1536, 2048)])
    return np.arange(512, 1536)


def shard_inputs(inputs):
    x = np.asarray(inputs["x"], np.float32)
    mask = np.asarray(inputs["mask"], bool)
    w = {k: np.asarray(inputs[k], np.float32) for k in
         ["wq", "bq", "wk", "bk", "wv", "bv", "wo", "bo", "g1", "be1",
          "w1", "b1", "w2", "b2", "g2", "be2"]}

    base = dict(
        wq=np.ascontiguousarray(w["wq"].astype(BF)),
        wk=np.ascontiguousarray(w["wk"].astype(BF)),
        wv=np.ascontiguousarray(w["wv"].astype(BF)),
        wo=np.ascontiguousarray(w["wo"].astype(BF)),
        w1=np.ascontiguousarray(w["w1"].astype(BF)),
        w2=np.ascontiguousarray(w["w2"].astype(ml_dtypes.float8_e4m3)),
        bqp=np.ascontiguousarray(w["bq"].reshape(HP, P).T),
        bkp=np.ascontiguousarray(w["bk"].reshape(HP, P).T),
        b1p=np.ascontiguousarray(w["b1"].reshape(DFC, P).T),
        g1r=np.ascontiguousarray(w["g1"].reshape(1, D)),
        be1r=np.ascontiguousarray(w["be1"].reshape(1, D)),
        g2r=np.ascontiguousarray(w["g2"].reshape(1, D)),
        be2r=np.ascontiguousarray(w["be2"].reshape(1, D)),
        b2r=np.ascontiguousarray(w["b2"].reshape(1, D)),
    )
    # bv and bo fold into the residual: ctx@wo + bo + x with v-bias bv adds
    # a constant row bv@wo (softmax rows sum to 1)
    res_const = (w["bo"] + w["bv"] @ w["wo"]).astype(np.float32)

    in_maps = []
    for c in range(N_CORES):
        b, half = divmod(c, 2)
        qr = _qrows(half)
        xb = x[b]
        xq = xb[qr]
        m = dict(base)
        m["xT"] = np.ascontiguousarray(xb.T.astype(BF))
        m["xqT"] = np.ascontiguousarray(xq.T.astype(BF))
        m["xres"] = np.ascontiguousarray(xq + res_const[None, :])
        m["maskT"] = np.ascontiguousarray((~mask[b][qr]).T.astype(BF))
        in_maps.append(m)
    return in_maps


def gather_outputs(results):
    y = np.empty((B, S, D), np.float32)
    for c in range(N_CORES):
        b, half = divmod(c, 2)
        y[b, _qrows(half)] = results[c]["out"]
    return y


def kernel(**inputs):
    nc = _get_program()
    in_maps = shard_inputs(inputs)
    res = run_bass_kernel_spmd(nc, in_maps, list(range(N_CORES)))
    return gather_outputs(res.results)


if __name__ == "__main__":
    build_program()
    print("program built ok")



# revision 57
# speedup vs baseline: 1.0015x; 1.0015x over previous
"""Trainium2 Bass kernel for a dense transformer decoder block.

Reference computation (B=4, S=2048, D=768, H=12, DK=64, DF=3072):
    q,k,v = x@wq+bq, x@wk+bk, x@wv+bv          (per-head split, DK=64)
    attn  = softmax(mask(q k^T / 8))
    ctx   = attn @ v
    h     = LN(ctx@wo + bo + x; g1, be1)
    out   = LN(gelu_exact(h@w1 + b1)@w2 + b2 + h; g2, be2)

Sharding: pure data parallel, zero collectives. 8 cores = 4 batch elements
x 2 query groups of 1024 rows. Queries are paired so each core's two
512-row query blocks need key extents {<=8, <=16} key-blocks of 128
(block-causal skip); the exact mask is applied as data.
Core 2b+0: query rows [0:512) u [1536:2048) of batch b.
Core 2b+1: query rows [512:1536) of batch b.
Every core runs the identical SPMD program; per-core behavior differs only
through input data (sliced/transposed/cast on the host).

Schedule: attention is ACT-(exp)-bound, so independent PE work is woven
between attention iterations to keep the tensor engine dense (and its HAM
clock warm): the sb2/sb3 K,V projections run under qb0 attention, and the
qb0 out-projection + LN1 + h-transposes run under qb1 attention.
"""

from contextlib import ExitStack

import numpy as np
import ml_dtypes

import concourse.bass as bass
import concourse.tile as tile
from concourse import bacc, mybir
from concourse.bass_utils import run_bass_kernel_spmd
from concourse.masks import make_identity

F32 = mybir.dt.float32
I32 = mybir.dt.int32
BF16 = mybir.dt.bfloat16
F8E4 = mybir.dt.float8e4
AF = mybir.ActivationFunctionType
OP = mybir.AluOpType
BF = ml_dtypes.bfloat16

B, S, D, H, DK = 4, 2048, 768, 12, 64
DF = 4 * D
EPS = 1e-5
P = 128
SQ = 1024            # query rows per core
HP = H // 2          # 6 head pairs
KB = S // P          # 16 key blocks
QB = 2               # query blocks of 512 per core
QBS = 512
EXT = [8, 16]        # key-block extent per query block (block-causal skip)
DC = D // P          # 6 chunks of the model dim
DFC = DF // P        # 24 chunks of the FFN dim
QC = SQ // P         # 8 query chunks of 128
NH = 2               # 384-wide halves of D for PSUM-friendly matmul N
NHW = D // NH        # 384
SB = S // QBS        # 4 key column slabs

N_CORES = 8


def emit(ctx: ExitStack, tc: tile.TileContext, io: dict):
    nc = tc.nc

    xT, xqT, xres, maskT = io["xT"], io["xqT"], io["xres"], io["maskT"]
    wq, wk, wv, wo, w1, w2 = io["wq"], io["wk"], io["wv"], io["wo"], io["w1"], io["w2"]
    out = io["out"]

    # ---- constants ----------------------------------------------------
    const = ctx.enter_context(tc.tile_pool(name="const", bufs=1))
    ident = const.tile([P, P], BF16)
    make_identity(nc, ident)
    eps_t = const.tile([P, 1], F32)
    nc.vector.memset(eps_t, EPS)

    # PE warmup: the first input DMA bytes land ~9us in and the first
    # projection matmul issues ~22us in; without sustained PE activity the
    # HAM clock gate holds the array at 1.2GHz for the first ~3.4us of real
    # work. Junk matmuls on the on-chip identity span the DMA wait so the
    # projections start at 2.4GHz.
    warm_ps = tc.alloc_tile_pool(name="warm_ps", bufs=1, space="PSUM")
    wps = warm_ps.tile([64, 64], F32)
    for _ in range(120):
        nc.tensor.matmul(wps, lhsT=ident[:, 0:64], rhs=ident[:, 0:64],
                         start=True, stop=True)
    warm_ps.release()

    bqp = const.tile([P, HP], F32)
    nc.gpsimd.dma_start(out=bqp, in_=io["bqp"])
    bkp = const.tile([P, HP], F32)
    nc.gpsimd.dma_start(out=bkp, in_=io["bkp"])
    b1p = const.tile([P, DFC], F32)

    def brow_alloc(name):
        # [1, D] dram tensor broadcast-DMA'd across 128 partitions; tiles
        # allocated here, transfers issued after the projection inputs so
        # the first matmuls aren't queued behind 1.9MB of LN constants
        return const.tile([P, D], F32, tag=name, name=name)

    g1b, be1b, g2b, be2b, b2b = map(
        brow_alloc, ["g1r", "be1r", "g2r", "be2r", "b2r"])
    _brow_tiles = {"g1r": g1b, "be1r": be1b, "g2r": g2b, "be2r": be2b,
                   "b2r": b2b}

    def issue_ln_const_dmas():
        nc.gpsimd.dma_start(out=b1p, in_=io["b1p"])
        for name, t in _brow_tiles.items():
            a = io[name]
            src = bass.AP(tensor=a.tensor, offset=a.offset,
                          ap=[[0, P]] + list(a.ap[1:]))
            nc.gpsimd.dma_start(out=t, in_=src)

    # ---- FFN-phase tensors: left stack, below attn_in so release order
    # stays LIFO (h/hT are written during the attention epilogue fillers)
    ffn = tc.alloc_tile_pool(name="ffn", bufs=1)
    h_sb = ffn.tile([P, QC, D], BF16)     # LN1 out (residual + FFN rhs)
    hT = ffn.tile([P, DC, SQ], BF16)
    ln_wk = tc.alloc_tile_pool(name="ln_wk", bufs=1)

    # ---- attention inputs (live through attention) --------------------
    attn_in = tc.alloc_tile_pool(name="attn_in", bufs=1)
    KT = attn_in.tile([P, HP, S], BF16)            # K^T, head pairs on partitions
    Vaug = attn_in.tile([P, KB, H, DK + 1], BF16)  # V + ones column per head
    QT = attn_in.tile([P, HP, SQ], BF16)
    # only the mask quadrants that aren't structurally skipped:
    # qb0 masks key blocks 0..7, qb1 masks key blocks 8..15
    mT1 = attn_in.tile([P, 8, QBS], BF16)
    mr = maskT.rearrange("(kb p) q -> p kb q", p=P)
    nc.vector.memset(Vaug[:, :, :, DK : DK + 1], 1.0)

    # ---- post-attention inputs (right-side stack, phase-scoped) -------
    mid_ctx = tc.alloc_tile_pool(name="mid_ctx", bufs=1, side="right")
    ctxT = mid_ctx.tile([P, DC, SQ], BF16)

    kv_in = tc.alloc_tile_pool(name="kv_in", bufs=1, side="right")
    wk_sb = kv_in.tile([P, DC, D], BF16)
    wv_sb = kv_in.tile([P, DC, D], BF16)
    xT23 = kv_in.tile([P, DC, S // 2], BF16)
    xt01p = tc.alloc_tile_pool(name="xt01p", bufs=1, side="right")
    xT01 = xt01p.tile([P, DC, S // 2], BF16)
    xTr = xT.rearrange("(c p) s -> p c s", p=P)

    def xT_at(sb):
        t = xT01 if sb < 2 else xT23
        return t, (sb % 2) * QBS

    q_in = tc.alloc_tile_pool(name="q_in", bufs=1, side="right")
    wq_sb = q_in.tile([P, DC, D], BF16)
    xqT_sb = q_in.tile([P, DC, SQ], BF16)
    # wq lands as 6 per-head-pair column slices: q_unit(hp0) only needs
    # slice 0 (200KB), so the first projection starts ~9us earlier than
    # waiting for the whole 1.2MB transfer
    wqr = wq.rearrange("(c p) n -> p c n", p=P)
    for hp in range(HP):
        nc.sync.dma_start(out=wq_sb[:, :, hp * P : (hp + 1) * P],
                          in_=wqr[:, :, hp * P : (hp + 1) * P])
    xqr = xqT.rearrange("(c p) s -> p c s", p=P)
    for sb in range(2):
        nc.scalar.dma_start(out=xqT_sb[:, :, sb * QBS : (sb + 1) * QBS],
                            in_=xqr[:, :, sb * QBS : (sb + 1) * QBS])
    for sb in range(SB):
        dst = xT01 if sb < 2 else xT23
        nc.sync.dma_start(out=dst[:, :, (sb % 2) * QBS : (sb % 2 + 1) * QBS],
                          in_=xTr[:, :, sb * QBS : (sb + 1) * QBS])
    # wk on the gpsimd queue, wv on scalar: spreads the projection-input
    # load across three DGE queues instead of two
    for c in range(DC):
        nc.gpsimd.dma_start(out=wk_sb[:, c, :],
                            in_=wk.rearrange("(c p) n -> p c n", p=P)[:, c, :])
    for c in range(DC):
        nc.gpsimd.dma_start(out=wv_sb[:, c, :],
                            in_=wv.rearrange("(c p) n -> p c n", p=P)[:, c, :])
    # mask + LN constants: not needed until well after the projections,
    # so these transfers queue behind the weight/x loads
    issue_ln_const_dmas()
    nc.gpsimd.dma_start(out=mT1, in_=mr[:, 8:16, QBS : 2 * QBS])

    def layer_norm(wk_pool, src, gb, bb, dst, act_stats=False):
        # LN over the free dim (768) of fp32 src [128, 768]; dst may be the
        # same tile or a bf16 target
        mv = wk_pool.tile([P, 2], F32, tag="mv", bufs=3)
        if act_stats:
            # stats on ACT (idle in the FFN phase): two accumulating passes
            # give sum(x) and sum(x^2); relieves the DVE-bound LN2 drain
            # that follows the last FFN matmuls
            scr = wk_pool.tile([P, D], F32, tag="lnscr", bufs=2)
            s1 = wk_pool.tile([P, 1], F32, tag="s1", bufs=3)
            s2 = wk_pool.tile([P, 1], F32, tag="s2", bufs=3)
            nc.scalar.activation(out=scr, in_=src, func=AF.Square,
                                 accum_out=s2)
            nc.scalar.activation(out=scr, in_=src, func=AF.Identity,
                                 accum_out=s1)
            nc.vector.tensor_scalar_mul(out=mv[:, 0:1], in0=s1,
                                        scalar1=1.0 / D)
            musq = wk_pool.tile([P, 1], F32, tag="musq", bufs=3)
            nc.vector.tensor_tensor(out=musq, in0=mv[:, 0:1],
                                    in1=mv[:, 0:1], op=OP.mult)
            nc.vector.tensor_scalar(out=mv[:, 1:2], in0=s2,
                                    scalar1=1.0 / D, scalar2=musq,
                                    op0=OP.mult, op1=OP.subtract)
        else:
            stats = wk_pool.tile([P, 2, 6], F32, tag="stats", bufs=3)
            for j in range(2):
                nc.vector.bn_stats(out=stats[:, j, :],
                                   in_=src[:, j * 384 : (j + 1) * 384])
            nc.vector.bn_aggr(out=mv, in_=stats)
        # rstd = rsqrt(var+eps) entirely on DVE (Quake seed + 3 Newton
        # steps, ~1e-7 rel err): an ACT Sqrt here would force a ~2.7us
        # activation-table swap against the attention exps / FFN gelus
        vh = wk_pool.tile([P, 1], F32, tag="vh", bufs=3)
        nc.vector.tensor_scalar(out=vh, in0=mv[:, 1:2], scalar1=EPS,
                                scalar2=-0.5, op0=OP.add, op1=OP.mult)
        seed = wk_pool.tile([P, 1], I32, tag="seed", bufs=3)
        nc.vector.tensor_scalar(out=seed, in0=mv[:, 1:2].bitcast(I32),
                                scalar1=1, scalar2=None,
                                op0=OP.logical_shift_right)
        nc.vector.tensor_scalar(out=seed, in0=seed, scalar1=-1,
                                scalar2=0x5F3759DF, op0=OP.mult, op1=OP.add)
        rstd = wk_pool.tile([P, 1], F32, tag="rstd", bufs=3)
        y = seed.bitcast(F32)
        for _ in range(3):
            w = wk_pool.tile([P, 1], F32, tag="nw", bufs=3)
            nc.vector.scalar_tensor_tensor(out=w, in0=y, scalar=y,
                                           in1=vh, op0=OP.mult, op1=OP.mult)
            nc.vector.tensor_scalar(out=w, in0=w, scalar1=1.5, scalar2=None,
                                    op0=OP.add)
            nc.vector.tensor_tensor(out=rstd, in0=y, in1=w, op=OP.mult)
            y = rstd
        nc.vector.tensor_scalar_sub(out=src, in0=src, scalar1=mv[:, 0:1])
        nc.vector.scalar_tensor_tensor(out=src, in0=src, scalar=rstd,
                                       in1=gb, op0=OP.mult, op1=OP.mult)
        nc.vector.tensor_tensor(out=dst, in0=src, in1=bb, op=OP.add)

    proj_ps = tc.alloc_tile_pool(name="proj_ps", bufs=2, space="PSUM", side="right")
    with tc.tile_pool(name="sc_ps", bufs=2, space="PSUM") as sc_ps, \
         tc.tile_pool(name="cx_ps", bufs=1, space="PSUM") as cx_ps:
        # at_sb/nm_sb/mT0 are allocated only once the q-projection inputs are
        # released -- their SBUF footprints must not overlap
        pools = {}

        # ---------- projection work units ----------
        def q_unit(hp, sb):
            ps = proj_ps.tile([P, QBS], F32, tag="proj")
            for c in range(DC):
                nc.tensor.matmul(
                    ps, lhsT=wq_sb[:, c, hp * P : (hp + 1) * P],
                    rhs=xqT_sb[:, c, sb * QBS : (sb + 1) * QBS],
                    start=(c == 0), stop=(c == DC - 1),
                )
            # bias on DVE: the ACT FIFO is clogged with DMA-descriptor and
            # semaphore instructions at startup, which left the PE stalled
            # ~8us on the proj_ps ring waiting for the first bias reads
            nc.vector.tensor_scalar_add(
                out=QT[:, hp, sb * QBS : (sb + 1) * QBS], in0=ps,
                scalar1=bqp[:, hp : hp + 1],
            )

        def k_unit(hp, sb, on_act=True):
            xt, off = xT_at(sb)
            ps = proj_ps.tile([P, QBS], F32, tag="proj")
            for c in range(DC):
                nc.tensor.matmul(
                    ps, lhsT=wk_sb[:, c, hp * P : (hp + 1) * P],
                    rhs=xt[:, c, off : off + QBS],
                    start=(c == 0), stop=(c == DC - 1),
                )
            if on_act:
                nc.scalar.activation(
                    out=KT[:, hp, sb * QBS : (sb + 1) * QBS], in_=ps,
                    func=AF.Identity, bias=bkp[:, hp : hp + 1],
                )
            else:
                # inside the attention interleave ACT is the bottleneck chain
                nc.vector.tensor_scalar_add(
                    out=KT[:, hp, sb * QBS : (sb + 1) * QBS], in0=ps,
                    scalar1=bkp[:, hp : hp + 1],
                )

        def v_unit(kb, nh, on_act=True):
            xt, off = xT_at(kb // (QBS // P))
            kb_off = off // P + kb % (QBS // P)
            ps = proj_ps.tile([P, QBS], F32, tag="proj")
            psv = ps[:, 0:NHW]
            for c in range(DC):
                nc.tensor.matmul(
                    psv, lhsT=xt[:, c, kb_off * P : (kb_off + 1) * P],
                    rhs=wv_sb[:, c, nh * NHW : (nh + 1) * NHW],
                    start=(c == 0), stop=(c == DC - 1),
                )
            if on_act:
                nc.scalar.activation(
                    out=Vaug[:, kb, nh * 6 : (nh + 1) * 6, 0:DK],
                    in_=psv.rearrange("p (h d) -> p h d", d=DK),
                    func=AF.Copy,
                )
            else:
                nc.vector.tensor_copy(
                    out=Vaug[:, kb, nh * 6 : (nh + 1) * 6, 0:DK],
                    in_=psv.rearrange("p (h d) -> p h d", d=DK),
                )

        def kv_slab(sb, on_act=True):
            for hp in range(HP):
                k_unit(hp, sb, on_act)
            for j in range(QBS // P):
                for nh in range(NH):
                    v_unit(sb * (QBS // P) + j, nh, on_act)

        # ---------- attention iteration ----------
        pending = []

        def make_norm(cxs_e, cxs_o, den2, hp, qs):
            def go():
                # one reciprocal serves both heads: its cost scales with the
                # free size, not the partition count
                rec2 = pools['nm_sb'].tile([DK + 1, QBS], F32, tag="rec2", bufs=1)
                # softmax denominators are well-conditioned (>=1, <~4e3):
                # the ~5x faster 18-bit approx is far inside tolerance
                nc.vector.reciprocal_approx_fast(rec2, den2)
                # partition_broadcast replicates the tile's physical partition
                # 0, so the head-odd reciprocal must move to its own base-0
                # tile first
                rec_o = pools['nm_sb'].tile([1, QBS], F32, tag="rec_o", bufs=1)
                nc.vector.tensor_copy(out=rec_o, in_=rec2[DK : DK + 1, :])
                for i, (cxs, pb) in enumerate(((cxs_e, 0), (cxs_o, DK))):
                    src_r = rec2[0:1, :] if i == 0 else rec_o[0:1, :]
                    den_b = pools['nm_sb'].tile([DK, QBS], F32, tag="den_b", bufs=1)
                    nc.gpsimd.partition_broadcast(den_b, src_r)
                    nc.vector.tensor_tensor(
                        out=ctxT[pb : pb + DK, hp, qs], in0=cxs[0:DK, :],
                        in1=den_b, op=OP.mult,
                    )
            return go

        def attn_iter(hp, qb, fill=None):
            ext = EXT[qb]
            qs = slice(qb * QBS, (qb + 1) * QBS)
            cx_e = cx_ps.tile([DK + 1, QBS], F32, tag="cx_e")
            cx_o = cx_ps.tile([DK + 1, QBS], F32, tag="cx_o")
            def front(gb):
                # scores + exp (+ mask) for key-block pair gb
                pt = pools['at_sb'].tile([P, 2, 2, QBS], BF16, tag="pt")
                for gi in range(2):
                    g = gb + gi
                    ks = slice(g * P, (g + 1) * P)
                    sc = sc_ps.tile([P, 2, QBS], F32, tag="sc")
                    # the two heads of a pair hit disjoint PE row groups and
                    # run concurrently in the array
                    nc.tensor.matmul(sc[:, 0, :], lhsT=KT[0:DK, hp, ks],
                                     rhs=QT[0:DK, hp, qs], start=True, stop=True)
                    nc.tensor.matmul(sc[:, 1, :], lhsT=KT[DK:P, hp, ks],
                                     rhs=QT[DK:P, hp, qs], start=True, stop=True)
                    nc.scalar.activation(out=pt[:, :, gi, :], in_=sc,
                                         func=AF.Exp, scale=1.0 / 8.0)
                # qb0 masks kb 0..7 via mT0; qb1 masks only kb 8..15 (below
                # the diagonal for every core) via mT1
                if qb == 0 or gb >= 8:
                    mq = pools['mT0'][:, gb : gb + 2, :] if qb == 0 else \
                        mT1[:, gb - 8 : gb - 6, :]
                    # one multiply for both heads: the mask AP broadcasts
                    # over the head dim with a 0-stride
                    mqb = bass.AP(tensor=mq.tensor, offset=mq.offset,
                                  ap=[mq.ap[0], [0, 2]] + list(mq.ap[1:]))
                    nc.vector.tensor_tensor(out=pt, in0=pt, in1=mqb,
                                            op=OP.mult)
                return pt

            def back(gb, pt):
                for gi in range(2):
                    g = gb + gi
                    nc.tensor.matmul(cx_e, lhsT=Vaug[:, g, 2 * hp, :],
                                     rhs=pt[:, 0, gi, :],
                                     start=(g == 0), stop=(g == ext - 1))
                    nc.tensor.matmul(cx_o, lhsT=Vaug[:, g, 2 * hp + 1, :],
                                     rhs=pt[:, 1, gi, :],
                                     start=(g == 0), stop=(g == ext - 1))
                if gb == 2 and pending:
                    pending.pop()()
                if fill and (qb == 0 or gb % 4 == 2):
                    fill.pop(0)()

            # one-pair software pipeline: pair t+1's score matmuls issue
            # BEFORE pair t's ctx matmuls/fillers, so the exp chain on ACT
            # (the qb1 bottleneck) never starves behind PE queue order --
            # S(t,a) S(t,b) C(t-1) takes ~1.7us, just past exp(t,a)'s 1.57us
            prev = None
            for gb in range(0, ext, 2):
                pt = front(gb)
                if prev is not None:
                    back(*prev)
                prev = (gb, pt)
            back(*prev)
            # stage ctx to SBUF immediately: frees the PSUM bank within one
            # DVE copy so the cx pool gets away with a single buffer
            cxs_e = pools['nm_sb'].tile([DK + 1, QBS], F32, tag="cxs_e")
            nc.vector.tensor_copy(out=cxs_e, in_=cx_e)
            cxs_o = pools['nm_sb'].tile([DK + 1, QBS], F32, tag="cxs_o")
            nc.vector.tensor_copy(out=cxs_o, in_=cx_o)
            den2 = pools['nm_sb'].tile([DK + 1, QBS], F32, tag="den2")
            nc.vector.memset(den2, 1.0)
            nc.vector.tensor_copy(out=den2[0:1, :], in_=cx_e[DK : DK + 1, :])
            nc.vector.tensor_copy(out=den2[DK : DK + 1, :], in_=cx_o[DK : DK + 1, :])
            pending.append(make_norm(cxs_e, cxs_o, den2, hp, qs))

        # ---------- schedule: projections + qb0 attention ----------
        for hp in range(HP):
            q_unit(hp, 0)
        kv_slab(0, on_act=False)
        kv_slab(1, on_act=False)
        for hp in range(HP):
            q_unit(hp, 1)
        q_in.release()
        xt01p.release()
        pools['at_sb'] = tc.alloc_tile_pool(name="at_sb", bufs=3)
        pools['nm_sb'] = tc.alloc_tile_pool(name="nm_sb", bufs=2)
        mT0p = tc.alloc_tile_pool(name="mT0p", bufs=1)
        pools['mT0'] = mT0p.tile([P, 8, QBS], BF16, name="mT0", tag="mT0")
        nc.gpsimd.dma_start(out=pools['mT0'], in_=mr[:, 0:8, 0:QBS])
        # on_act=False: these run woven into qb0 attention where ACT is the
        # exp-bound critical chain -- their epilogues go to DVE instead
        kv_fill = [(lambda hp=hp, sb=sb: k_unit(hp, sb, on_act=False))
                   for sb in (2, 3) for hp in range(HP)] + \
                  [(lambda kb=kb, nh=nh: v_unit(kb, nh, on_act=False))
                   for kb in range(8, KB) for nh in range(NH)]
        for hp in range(HP):
            attn_iter(hp, 0, kv_fill)
        for fn in kv_fill:
            fn()
        kv_fill.clear()
        kv_in.release()
        proj_ps.release()
        mT0p.release()

        # ---------- qb1 attention with qb0 epilogue woven in ----------
        mid_ow = tc.alloc_tile_pool(name="mid_ow", bufs=1, side="right")
        xres_sb = mid_ow.tile([P, QC, D], F32)
        nc.gpsimd.dma_start(out=xres_sb,
                            in_=xres.rearrange("(c p) n -> p c n", p=P))
        wo_sb = mid_ow.tile([P, DC, D], BF16)
        nc.gpsimd.dma_start(out=wo_sb, in_=wo.rearrange("(c p) n -> p c n", p=P))
        op_ps = tc.alloc_tile_pool(name="op_ps", bufs=1, space="PSUM", side="right")
        tp_ps = tc.alloc_tile_pool(name="tp_ps", bufs=1, space="PSUM", side="right")

        hpre_map = {}

        def op_half(qc, nh):
            def go():
                if qc not in hpre_map:
                    hpre_map[qc] = ln_wk.tile([P, D], F32, tag="hpre",
                                              bufs=3, name=f"hpre_{qc}")
                hpre = hpre_map[qc]
                ps = op_ps.tile([P, NHW], F32, tag="op")
                for c in range(DC):
                    nc.tensor.matmul(
                        ps, lhsT=ctxT[:, c, qc * P : (qc + 1) * P],
                        rhs=wo_sb[:, c, nh * NHW : (nh + 1) * NHW],
                        start=(c == 0), stop=(c == DC - 1),
                    )
                nc.vector.scalar_tensor_tensor(
                    out=hpre[:, nh * NHW : (nh + 1) * NHW], in0=ps,
                    scalar=1.0, in1=xres_sb[:, qc, nh * NHW : (nh + 1) * NHW],
                    op0=OP.mult, op1=OP.add,
                )
            return go

        def ln_unit(qc):
            def go():
                layer_norm(ln_wk, hpre_map.pop(qc), g1b, be1b, h_sb[:, qc, :])
            return go

        def transp_half(qc, lo):
            def go():
                for c in range(lo, lo + DC // 2):
                    tp = tp_ps.tile([P, P], BF16, tag="tp")
                    nc.tensor.transpose(tp, h_sb[:, qc, c * P : (c + 1) * P],
                                        ident)
                    nc.scalar.activation(out=hT[:, c, qc * P : (qc + 1) * P],
                                         in_=tp, func=AF.Copy)
            return go

        def outproj_unit(qc):
            def go():
                op_half(qc, 0)()
                op_half(qc, 1)()
                ln_unit(qc)()
            return go

        def transp_unit(qc):
            def go():
                for c in range(DC):
                    tp = tp_ps.tile([P, P], BF16, tag="tp")
                    nc.tensor.transpose(tp, h_sb[:, qc, c * P : (c + 1) * P],
                                        ident)
                    nc.scalar.activation(out=hT[:, c, qc * P : (qc + 1) * P],
                                         in_=tp, func=AF.Copy)
            return go

        fillers = []
        for qc in range(4):
            fillers += [op_half(qc, 0), op_half(qc, 1), ln_unit(qc)]
        for qc in range(4):
            fillers += [transp_half(qc, 0), transp_half(qc, DC // 2)]
        for hp in range(HP):
            attn_iter(hp, 1, fillers)
        for fn in pending:
            fn()
        pending.clear()
        for fn in fillers:
            fn()
        pools['nm_sb'].release()
        pools['at_sb'].release()

    attn_in.release()

    # prefetch the 9.4MB w1/w2 load so it overlaps the qc4-7 epilogue
    # instead of stalling the FFN phase start
    w12_in = tc.alloc_tile_pool(name="w12_in", bufs=1)
    w1_sb = w12_in.tile([P, DC, DF], BF16)
    w1r = w1.rearrange("(c p) n -> p c n", p=P)
    nc.sync.dma_start(out=w1_sb[:, 0:3, :], in_=w1r[:, 0:3, :])
    nc.scalar.dma_start(out=w1_sb[:, 3:6, :], in_=w1r[:, 3:6, :])
    w2_sb = w12_in.tile([P, DFC, D], F8E4)
    nc.gpsimd.dma_start(out=w2_sb, in_=w2.rearrange("(c p) n -> p c n", p=P))

    # ====== FFN: f1^T = gelu(w1^T h^T + b1); out = LN2(f1g^T w2 + h) ====
    with tc.tile_pool(name="f1_ps", bufs=3, space="PSUM") as f1_ps, \
         tc.tile_pool(name="f2_ps", bufs=3, space="PSUM") as f2_ps, \
         tc.tile_pool(name="f1g_sb", bufs=2) as f1g_sb, \
         tc.tile_pool(name="out_sb", bufs=3) as out_sb:

        def ffn1(qb, fill=None):
            qs = slice(qb * QBS, (qb + 1) * QBS)
            # fp8: FC2 runs as DoubleRow (2 k-tiles/pass, ~1.4x); gelu
            # outputs |g|<~8 and w2~0.02 are far inside e4m3 range, and the
            # FC2-only quantization error (~1.4e-2) fits the 2e-2 tolerance
            f1g = f1g_sb.tile([P, DFC, QBS], F8E4, tag="f1g",
                              name=f"f1g_{qb}")
            for f in range(DFC):
                ps = f1_ps.tile([P, QBS], F32, tag="f1")
                for c in range(DC):
                    nc.tensor.matmul(
                        ps, lhsT=w1_sb[:, c, f * P : (f + 1) * P],
                        rhs=hT[:, c, qs], start=(c == 0), stop=(c == DC - 1),
                    )
                nc.scalar.activation(out=f1g[:, f, :], in_=ps, func=AF.Gelu,
                                     bias=b1p[:, f : f + 1])
                if fill:
                    fill.pop(0)()
            return f1g

        def ffn2(qb, f1g):
            for sq in range(QBS // P):
                qc = qb * (QBS // P) + sq
                ot = out_sb.tile([P, D], F32, tag="ot")
                for nh in range(NH):
                    ps = f2_ps.tile([P, NHW], F32, tag="f2")
                    for fp in range(DFC // 2):
                        nc.tensor.matmul(
                            ps,
                            lhsT=f1g[:, 2 * fp : 2 * fp + 2,
                                     sq * P : (sq + 1) * P],
                            rhs=w2_sb[:, 2 * fp : 2 * fp + 2,
                                      nh * NHW : (nh + 1) * NHW],
                            start=(fp == 0), stop=(fp == DFC // 2 - 1),
                            perf_mode=mybir.MatmulPerfMode.DoubleRow,
                        )
                    nc.vector.scalar_tensor_tensor(
                        out=ot[:, nh * NHW : (nh + 1) * NHW], in0=ps,
                        scalar=1.0,
                        in1=h_sb[:, qc, nh * NHW : (nh + 1) * NHW],
                        op0=OP.mult, op1=OP.add,
                    )
                nc.vector.tensor_tensor(out=ot, in0=ot, in1=b2b, op=OP.add)
                # act_stats=True regressed: ACT is strict-FIFO, so a stats
                # pass waiting on the DVE-built ot blocks later gelus
                layer_norm(ln_wk, ot, g2b, be2b, ot)
                nc.sync.dma_start(out=out[qc * P : (qc + 1) * P, :], in_=ot)

        # ---- qc4-7 out-projection epilogue woven INTO FFN1(qb0): FFN1 qb0
        # only needs hT qc0-3 (built during the qb1 fillers). Each op_half
        # rides between two dense f-units so its bufs=1 PSUM ring drains
        # behind PE work instead of serializing against the DVE adds
        epi = []
        for qc in range(4, QC):
            epi += [op_half(qc, 0), op_half(qc, 1), ln_unit(qc)]
        f1g0 = ffn1(0, epi)
        for qc in range(4, QC):
            transp_unit(qc)()
        ffn2(0, f1g0)
        ffn2(1, ffn1(1))

    tp_ps.release()
    op_ps.release()
    mid_ow.release()
    mid_ctx.release()

    w12_in.release()
    ln_wk.release()
    ffn.release()


def build_program():
    nc = bacc.Bacc("TRN2", target_bir_lowering=False, debug=False,
                   enable_asserts=False, num_devices=N_CORES)
    io = {}

    def din(name, shape, dt):
        io[name] = nc.dram_tensor(name, list(shape), dt, kind="ExternalInput").ap()

    din("xT", (D, S), BF16)
    din("xqT", (D, SQ), BF16)
    din("xres", (SQ, D), F32)
    din("maskT", (S, SQ), BF16)
    din("wq", (D, D), BF16)
    din("wk", (D, D), BF16)
    din("wv", (D, D), BF16)
    din("wo", (D, D), BF16)
    din("w1", (D, DF), BF16)
    din("w2", (DF, D), F8E4)
    din("bqp", (P, HP), F32)
    din("bkp", (P, HP), F32)
    din("b1p", (P, DFC), F32)
    for n in ["g1r", "be1r", "g2r", "be2r", "b2r"]:
        din(n, (1, D), F32)
    io["out"] = nc.dram_tensor("out", [SQ, D], F32, kind="ExternalOutput").ap()

    with tile.TileContext(nc) as tc:
        with ExitStack() as ctx:
            emit(ctx, tc, io)
    nc.compile()
    return nc


_NC = None


def _get_program():
    global _NC
    if _NC is None:
        _NC = build_program()
    return _NC


def _qrows(half):
    if half == 0:
        return np.concatenate([np.arange(0, 512), np.arange(1536, 2048)])
    return np.arange(512, 1536)


def shard_inputs(inputs):
    x = np.asarray(inputs["x"], np.float32)
    mask = np.asarray(inputs["mask"], bool)
    w = {k: np.asarray(inputs[k], np.float32) for k in
         ["wq", "bq", "wk", "bk", "wv", "bv", "wo", "bo", "g1", "be1",
          "w1", "b1", "w2", "b2", "g2", "be2"]}

    base = dict(
        wq=np.ascontiguousarray(w["wq"].astype(BF)),
        wk=np.ascontiguousarray(w["wk"].astype(BF)),
        wv=np.ascontiguousarray(w["wv"].astype(BF)),
        wo=np.ascontiguousarray(w["wo"].astype(BF)),
        w1=np.ascontiguousarray(w["w1"].astype(BF)),
        w2=np.ascontiguousarray(w["w2"].astype(ml_dtypes.float8_e4m3)),
        bqp=np.ascontiguousarray(w["bq"].reshape(HP, P).T),
        bkp=np.ascontiguousarray(w["bk"].reshape(HP, P).T),
        b1p=np.ascontiguousarray(w["b1"].reshape(DFC, P).T),
        g1r=np.ascontiguousarray(w["g1"].reshape(1, D)),
        be1r=np.ascontiguousarray(w["be1"].reshape(1, D)),
        g2r=np.ascontiguousarray(w["g2"].reshape(1, D)),
        be2r=np.ascontiguousarray(w["be2"].reshape(1, D)),
        b2r=np.ascontiguousarray(w["b2"].reshape(1, D)),
    )
    # bv and bo fold into the residual: ctx@wo + bo + x with v-bias bv adds
    # a constant row bv@wo (softmax rows sum to 1)
    res_const = (w["bo"] + w["bv"] @ w["wo"]).astype(np.float32)

    in_maps = []
    for c in range(N_CORES):
        b, half = divmod(c, 2)
        qr = _qrows(half)
        xb = x[b]
        xq = xb[qr]
        m = dict(base)
        m["xT"] = np.ascontiguousarray(xb.T.astype(BF))
        m["xqT"] = np.ascontiguousarray(xq.T.astype(BF))
        m["xres"] = np.ascontiguousarray(xq + res_const[None, :])
        m["maskT"] = np.ascontiguousarray((~mask[b][qr]).T.astype(BF))
        in_maps.append(m)
    return in_maps


def gather_outputs(results):
    y = np.empty((B, S, D), np.float32)
    for c in range(N_CORES):
        b, half = divmod(c, 2)
        y[b, _qrows(half)] = results[c]["out"]
    return y


def kernel(**inputs):
    nc = _get_program()
    in_maps = shard_inputs(inputs)
    res = run_bass_kernel_spmd(nc, in_maps, list(range(N_CORES)))
    return gather_outputs(res.results)


if __name__ == "__main__":
    build_program()
    print("program built ok")



# revision 60
# speedup vs baseline: 1.0209x; 1.0194x over previous
"""Trainium2 Bass kernel for a dense transformer decoder block.

Reference computation (B=4, S=2048, D=768, H=12, DK=64, DF=3072):
    q,k,v = x@wq+bq, x@wk+bk, x@wv+bv          (per-head split, DK=64)
    attn  = softmax(mask(q k^T / 8))
    ctx   = attn @ v
    h     = LN(ctx@wo + bo + x; g1, be1)
    out   = LN(gelu_exact(h@w1 + b1)@w2 + b2 + h; g2, be2)

Sharding: pure data parallel, zero collectives. 8 cores = 4 batch elements
x 2 query groups of 1024 rows. Queries are paired so each core's two
512-row query blocks need key extents {<=8, <=16} key-blocks of 128
(block-causal skip); the exact mask is applied as data.
Core 2b+0: query rows [0:512) u [1536:2048) of batch b.
Core 2b+1: query rows [512:1536) of batch b.
Every core runs the identical SPMD program; per-core behavior differs only
through input data (sliced/transposed/cast on the host).

Schedule: attention is ACT-(exp)-bound, so independent PE work is woven
between attention iterations to keep the tensor engine dense (and its HAM
clock warm): the sb2/sb3 K,V projections run under qb0 attention, and the
qb0 out-projection + LN1 + h-transposes run under qb1 attention.
"""

from contextlib import ExitStack

import numpy as np
import ml_dtypes

import concourse.bass as bass
import concourse.tile as tile
from concourse import bacc, mybir
from concourse.bass_utils import run_bass_kernel_spmd
from concourse.masks import make_identity

F32 = mybir.dt.float32
I32 = mybir.dt.int32
BF16 = mybir.dt.bfloat16
F8E4 = mybir.dt.float8e4
AF = mybir.ActivationFunctionType
OP = mybir.AluOpType
BF = ml_dtypes.bfloat16

B, S, D, H, DK = 4, 2048, 768, 12, 64
DF = 4 * D
EPS = 1e-5
P = 128
SQ = 1024            # query rows per core
HP = H // 2          # 6 head pairs
KB = S // P          # 16 key blocks
QB = 2               # query blocks of 512 per core
QBS = 512
EXT = [8, 16]        # key-block extent per query block (block-causal skip)
DC = D // P          # 6 chunks of the model dim
DFC = DF // P        # 24 chunks of the FFN dim
QC = SQ // P         # 8 query chunks of 128
NH = 2               # 384-wide halves of D for PSUM-friendly matmul N
NHW = D // NH        # 384
SB = S // QBS        # 4 key column slabs

N_CORES = 8


def emit(ctx: ExitStack, tc: tile.TileContext, io: dict):
    nc = tc.nc

    xT, xqT, xres, maskT = io["xT"], io["xqT"], io["xres"], io["maskT"]
    wq, wk, wv, wo, w1, w2 = io["wq"], io["wk"], io["wv"], io["wo"], io["w1"], io["w2"]
    out = io["out"]

    # ---- constants ----------------------------------------------------
    const = ctx.enter_context(tc.tile_pool(name="const", bufs=1))
    ident = const.tile([P, P], BF16)
    make_identity(nc, ident)
    eps_t = const.tile([P, 1], F32)
    nc.vector.memset(eps_t, EPS)

    # PE warmup: the first input DMA bytes land ~9us in and the first
    # projection matmul issues ~22us in; without sustained PE activity the
    # HAM clock gate holds the array at 1.2GHz for the first ~3.4us of real
    # work. Junk matmuls on the on-chip identity span the DMA wait so the
    # projections start at 2.4GHz.
    warm_ps = tc.alloc_tile_pool(name="warm_ps", bufs=1, space="PSUM")
    wps = warm_ps.tile([64, 64], F32)
    for _ in range(340):
        nc.tensor.matmul(wps, lhsT=ident[:, 0:64], rhs=ident[:, 0:64],
                         start=True, stop=True)
    warm_ps.release()

    bqp = const.tile([P, HP], F32)
    nc.gpsimd.dma_start(out=bqp, in_=io["bqp"])
    bkp = const.tile([P, HP], F32)
    nc.gpsimd.dma_start(out=bkp, in_=io["bkp"])
    b1p = const.tile([P, DFC], F32)

    def brow_alloc(name):
        # [1, D] dram tensor broadcast-DMA'd across 128 partitions; tiles
        # allocated here, transfers issued after the projection inputs so
        # the first matmuls aren't queued behind 1.9MB of LN constants
        return const.tile([P, D], F32, tag=name, name=name)

    g1b, be1b, g2b, be2b, b2b = map(
        brow_alloc, ["g1r", "be1r", "g2r", "be2r", "b2r"])
    _brow_tiles = {"g1r": g1b, "be1r": be1b, "g2r": g2b, "be2r": be2b,
                   "b2r": b2b}

    def issue_ln_const_dmas():
        nc.gpsimd.dma_start(out=b1p, in_=io["b1p"])
        for name, t in _brow_tiles.items():
            a = io[name]
            src = bass.AP(tensor=a.tensor, offset=a.offset,
                          ap=[[0, P]] + list(a.ap[1:]))
            nc.gpsimd.dma_start(out=t, in_=src)

    # ---- FFN-phase tensors: left stack, below attn_in so release order
    # stays LIFO (h/hT are written during the attention epilogue fillers)
    ffn = tc.alloc_tile_pool(name="ffn", bufs=1)
    h_sb = ffn.tile([P, QC, D], BF16)     # LN1 out (residual + FFN rhs)
    hT = ffn.tile([P, DC, SQ], BF16)
    ln_wk = tc.alloc_tile_pool(name="ln_wk", bufs=1)

    # ---- attention inputs (live through attention) --------------------
    attn_in = tc.alloc_tile_pool(name="attn_in", bufs=1)
    KT = attn_in.tile([P, HP, S], BF16)            # K^T, head pairs on partitions
    Vaug = attn_in.tile([P, KB, H, DK + 1], BF16)  # V + ones column per head
    QT = attn_in.tile([P, HP, SQ], BF16)
    # only the mask quadrants that aren't structurally skipped:
    # qb0 masks key blocks 0..7, qb1 masks key blocks 8..15
    mT1 = attn_in.tile([P, 8, QBS], BF16)
    mr = maskT.rearrange("(kb p) q -> p kb q", p=P)
    nc.vector.memset(Vaug[:, :, :, DK : DK + 1], 1.0)

    # ---- post-attention inputs (right-side stack, phase-scoped) -------
    mid_ctx = tc.alloc_tile_pool(name="mid_ctx", bufs=1, side="right")
    ctxT = mid_ctx.tile([P, DC, SQ], BF16)

    kv_in = tc.alloc_tile_pool(name="kv_in", bufs=1, side="right")
    wk_sb = kv_in.tile([P, DC, D], BF16)
    wv_sb = kv_in.tile([P, DC, D], BF16)
    xT23 = kv_in.tile([P, DC, S // 2], BF16)
    xt01p = tc.alloc_tile_pool(name="xt01p", bufs=1, side="right")
    xT01 = xt01p.tile([P, DC, S // 2], BF16)
    xTr = xT.rearrange("(c p) s -> p c s", p=P)

    def xT_at(sb):
        t = xT01 if sb < 2 else xT23
        return t, (sb % 2) * QBS

    q_in = tc.alloc_tile_pool(name="q_in", bufs=1, side="right")
    wq_sb = q_in.tile([P, DC, D], BF16)
    xqT_sb = q_in.tile([P, DC, SQ], BF16)
    nc.sync.dma_start(out=wq_sb, in_=wq.rearrange("(c p) n -> p c n", p=P))
    xqr = xqT.rearrange("(c p) s -> p c s", p=P)
    for sb in range(2):
        nc.scalar.dma_start(out=xqT_sb[:, :, sb * QBS : (sb + 1) * QBS],
                            in_=xqr[:, :, sb * QBS : (sb + 1) * QBS])
    for sb in range(SB):
        dst = xT01 if sb < 2 else xT23
        nc.sync.dma_start(out=dst[:, :, (sb % 2) * QBS : (sb % 2 + 1) * QBS],
                          in_=xTr[:, :, sb * QBS : (sb + 1) * QBS])
    # wk on the gpsimd queue, wv on scalar: spreads the projection-input
    # load across three DGE queues instead of two
    for c in range(DC):
        nc.gpsimd.dma_start(out=wk_sb[:, c, :],
                            in_=wk.rearrange("(c p) n -> p c n", p=P)[:, c, :])
    for c in range(DC):
        nc.gpsimd.dma_start(out=wv_sb[:, c, :],
                            in_=wv.rearrange("(c p) n -> p c n", p=P)[:, c, :])
    # mask + LN constants: not needed until well after the projections,
    # so these transfers queue behind the weight/x loads
    issue_ln_const_dmas()
    nc.gpsimd.dma_start(out=mT1, in_=mr[:, 8:16, QBS : 2 * QBS])

    def layer_norm(wk_pool, src, gb, bb, dst):
        # LN over the free dim (768) of fp32 src [128, 768]; dst may be the
        # same tile or a bf16 target
        stats = wk_pool.tile([P, 2, 6], F32, tag="stats", bufs=3)
        for j in range(2):
            nc.vector.bn_stats(out=stats[:, j, :], in_=src[:, j * 384 : (j + 1) * 384])
        mv = wk_pool.tile([P, 2], F32, tag="mv", bufs=3)
        nc.vector.bn_aggr(out=mv, in_=stats)
        # rstd = rsqrt(var+eps) entirely on DVE (Quake seed + 3 Newton
        # steps, ~1e-7 rel err): an ACT Sqrt here would force a ~2.7us
        # activation-table swap against the attention exps / FFN gelus
        vh = wk_pool.tile([P, 1], F32, tag="vh", bufs=3)
        nc.vector.tensor_scalar(out=vh, in0=mv[:, 1:2], scalar1=EPS,
                                scalar2=-0.5, op0=OP.add, op1=OP.mult)
        seed = wk_pool.tile([P, 1], I32, tag="seed", bufs=3)
        nc.vector.tensor_scalar(out=seed, in0=mv[:, 1:2].bitcast(I32),
                                scalar1=1, scalar2=None,
                                op0=OP.logical_shift_right)
        nc.vector.tensor_scalar(out=seed, in0=seed, scalar1=-1,
                                scalar2=0x5F3759DF, op0=OP.mult, op1=OP.add)
        rstd = wk_pool.tile([P, 1], F32, tag="rstd", bufs=3)
        y = seed.bitcast(F32)
        for _ in range(3):
            w = wk_pool.tile([P, 1], F32, tag="nw", bufs=3)
            nc.vector.scalar_tensor_tensor(out=w, in0=y, scalar=y,
                                           in1=vh, op0=OP.mult, op1=OP.mult)
            nc.vector.tensor_scalar(out=w, in0=w, scalar1=1.5, scalar2=None,
                                    op0=OP.add)
            nc.vector.tensor_tensor(out=rstd, in0=y, in1=w, op=OP.mult)
            y = rstd
        nc.vector.tensor_scalar_sub(out=src, in0=src, scalar1=mv[:, 0:1])
        nc.vector.scalar_tensor_tensor(out=src, in0=src, scalar=rstd,
                                       in1=gb, op0=OP.mult, op1=OP.mult)
        nc.vector.tensor_tensor(out=dst, in0=src, in1=bb, op=OP.add)

    proj_ps = tc.alloc_tile_pool(name="proj_ps", bufs=2, space="PSUM", side="right")
    with tc.tile_pool(name="sc_ps", bufs=2, space="PSUM") as sc_ps, \
         tc.tile_pool(name="cx_ps", bufs=1, space="PSUM") as cx_ps:
        # at_sb/nm_sb/mT0 are allocated only once the q-projection inputs are
        # released -- their SBUF footprints must not overlap
        pools = {}

        # ---------- projection work units ----------
        def q_unit(hp, sb):
            ps = proj_ps.tile([P, QBS], F32, tag="proj")
            for c in range(DC):
                nc.tensor.matmul(
                    ps, lhsT=wq_sb[:, c, hp * P : (hp + 1) * P],
                    rhs=xqT_sb[:, c, sb * QBS : (sb + 1) * QBS],
                    start=(c == 0), stop=(c == DC - 1),
                )
            # bias on DVE: the ACT FIFO is clogged with DMA-descriptor and
            # semaphore instructions at startup, which left the PE stalled
            # ~8us on the proj_ps ring waiting for the first bias reads
            nc.vector.tensor_scalar_add(
                out=QT[:, hp, sb * QBS : (sb + 1) * QBS], in0=ps,
                scalar1=bqp[:, hp : hp + 1],
            )

        def k_unit(hp, sb, on_act=True):
            xt, off = xT_at(sb)
            ps = proj_ps.tile([P, QBS], F32, tag="proj")
            for c in range(DC):
                nc.tensor.matmul(
                    ps, lhsT=wk_sb[:, c, hp * P : (hp + 1) * P],
                    rhs=xt[:, c, off : off + QBS],
                    start=(c == 0), stop=(c == DC - 1),
                )
            if on_act:
                nc.scalar.activation(
                    out=KT[:, hp, sb * QBS : (sb + 1) * QBS], in_=ps,
                    func=AF.Identity, bias=bkp[:, hp : hp + 1],
                )
            else:
                # inside the attention interleave ACT is the bottleneck chain
                nc.vector.tensor_scalar_add(
                    out=KT[:, hp, sb * QBS : (sb + 1) * QBS], in0=ps,
                    scalar1=bkp[:, hp : hp + 1],
                )

        def v_unit(kb, nh, on_act=True):
            xt, off = xT_at(kb // (QBS // P))
            kb_off = off // P + kb % (QBS // P)
            ps = proj_ps.tile([P, QBS], F32, tag="proj")
            psv = ps[:, 0:NHW]
            for c in range(DC):
                nc.tensor.matmul(
                    psv, lhsT=xt[:, c, kb_off * P : (kb_off + 1) * P],
                    rhs=wv_sb[:, c, nh * NHW : (nh + 1) * NHW],
                    start=(c == 0), stop=(c == DC - 1),
                )
            if on_act:
                nc.scalar.activation(
                    out=Vaug[:, kb, nh * 6 : (nh + 1) * 6, 0:DK],
                    in_=psv.rearrange("p (h d) -> p h d", d=DK),
                    func=AF.Copy,
                )
            else:
                nc.vector.tensor_copy(
                    out=Vaug[:, kb, nh * 6 : (nh + 1) * 6, 0:DK],
                    in_=psv.rearrange("p (h d) -> p h d", d=DK),
                )

        def kv_slab(sb, on_act=True):
            for hp in range(HP):
                k_unit(hp, sb, on_act)
            for j in range(QBS // P):
                for nh in range(NH):
                    v_unit(sb * (QBS // P) + j, nh, on_act)

        # ---------- attention iteration ----------
        pending = []

        def make_norm(cxs_e, cxs_o, den2, hp, qs):
            def go():
                # one reciprocal serves both heads: its cost scales with the
                # free size, not the partition count
                rec2 = pools['nm_sb'].tile([DK + 1, QBS], F32, tag="rec2", bufs=1)
                # softmax denominators are well-conditioned (>=1, <~4e3):
                # the ~5x faster 18-bit approx is far inside tolerance
                nc.vector.reciprocal_approx_fast(rec2, den2)
                # partition_broadcast replicates the tile's physical partition
                # 0, so the head-odd reciprocal must move to its own base-0
                # tile first
                rec_o = pools['nm_sb'].tile([1, QBS], F32, tag="rec_o", bufs=1)
                nc.vector.tensor_copy(out=rec_o, in_=rec2[DK : DK + 1, :])
                for i, (cxs, pb) in enumerate(((cxs_e, 0), (cxs_o, DK))):
                    src_r = rec2[0:1, :] if i == 0 else rec_o[0:1, :]
                    den_b = pools['nm_sb'].tile([DK, QBS], F32, tag="den_b", bufs=1)
                    nc.gpsimd.partition_broadcast(den_b, src_r)
                    nc.vector.tensor_tensor(
                        out=ctxT[pb : pb + DK, hp, qs], in0=cxs[0:DK, :],
                        in1=den_b, op=OP.mult,
                    )
            return go

        def attn_iter(hp, qb, fill=None):
            ext = EXT[qb]
            qs = slice(qb * QBS, (qb + 1) * QBS)
            cx_e = cx_ps.tile([DK + 1, QBS], F32, tag="cx_e")
            cx_o = cx_ps.tile([DK + 1, QBS], F32, tag="cx_o")
            def front(gb):
                # scores + exp (+ mask) for key-block pair gb
                pt = pools['at_sb'].tile([P, 2, 2, QBS], BF16, tag="pt")
                for gi in range(2):
                    g = gb + gi
                    ks = slice(g * P, (g + 1) * P)
                    sc = sc_ps.tile([P, 2, QBS], F32, tag="sc")
                    # the two heads of a pair hit disjoint PE row groups and
                    # run concurrently in the array
                    nc.tensor.matmul(sc[:, 0, :], lhsT=KT[0:DK, hp, ks],
                                     rhs=QT[0:DK, hp, qs], start=True, stop=True)
                    nc.tensor.matmul(sc[:, 1, :], lhsT=KT[DK:P, hp, ks],
                                     rhs=QT[DK:P, hp, qs], start=True, stop=True)
                    nc.scalar.activation(out=pt[:, :, gi, :], in_=sc,
                                         func=AF.Exp, scale=1.0 / 8.0)
                # qb0 masks kb 0..7 via mT0; qb1 masks only kb 8..15 (below
                # the diagonal for every core) via mT1
                if qb == 0 or gb >= 8:
                    mq = pools['mT0'][:, gb : gb + 2, :] if qb == 0 else \
                        mT1[:, gb - 8 : gb - 6, :]
                    # one multiply for both heads: the mask AP broadcasts
                    # over the head dim with a 0-stride
                    mqb = bass.AP(tensor=mq.tensor, offset=mq.offset,
                                  ap=[mq.ap[0], [0, 2]] + list(mq.ap[1:]))
                    nc.vector.tensor_tensor(out=pt, in0=pt, in1=mqb,
                                            op=OP.mult)
                return pt

            def back(gb, pt):
                for gi in range(2):
                    g = gb + gi
                    nc.tensor.matmul(cx_e, lhsT=Vaug[:, g, 2 * hp, :],
                                     rhs=pt[:, 0, gi, :],
                                     start=(g == 0), stop=(g == ext - 1))
                    nc.tensor.matmul(cx_o, lhsT=Vaug[:, g, 2 * hp + 1, :],
                                     rhs=pt[:, 1, gi, :],
                                     start=(g == 0), stop=(g == ext - 1))
                if gb == 2 and pending:
                    pending.pop()()
                if fill and (qb == 0 or gb % 4 == 2):
                    fill.pop(0)()

            # one-pair software pipeline: pair t+1's score matmuls issue
            # BEFORE pair t's ctx matmuls/fillers, so the exp chain on ACT
            # (the qb1 bottleneck) never starves behind PE queue order --
            # S(t,a) S(t,b) C(t-1) takes ~1.7us, just past exp(t,a)'s 1.57us
            prev = None
            for gb in range(0, ext, 2):
                pt = front(gb)
                if prev is not None:
                    back(*prev)
                prev = (gb, pt)
            back(*prev)
            # stage ctx to SBUF immediately: frees the PSUM bank within one
            # DVE copy so the cx pool gets away with a single buffer
            cxs_e = pools['nm_sb'].tile([DK + 1, QBS], F32, tag="cxs_e")
            nc.vector.tensor_copy(out=cxs_e, in_=cx_e)
            cxs_o = pools['nm_sb'].tile([DK + 1, QBS], F32, tag="cxs_o")
            nc.vector.tensor_copy(out=cxs_o, in_=cx_o)
            den2 = pools['nm_sb'].tile([DK + 1, QBS], F32, tag="den2")
            nc.vector.memset(den2, 1.0)
            nc.vector.tensor_copy(out=den2[0:1, :], in_=cx_e[DK : DK + 1, :])
            nc.vector.tensor_copy(out=den2[DK : DK + 1, :], in_=cx_o[DK : DK + 1, :])
            pending.append(make_norm(cxs_e, cxs_o, den2, hp, qs))

        # ---------- schedule: projections + qb0 attention ----------
        for hp in range(HP):
            q_unit(hp, 0)
        kv_slab(0, on_act=False)
        kv_slab(1, on_act=False)
        for hp in range(HP):
            q_unit(hp, 1)
        q_in.release()
        xt01p.release()
        pools['at_sb'] = tc.alloc_tile_pool(name="at_sb", bufs=3)
        pools['nm_sb'] = tc.alloc_tile_pool(name="nm_sb", bufs=2)
        mT0p = tc.alloc_tile_pool(name="mT0p", bufs=1)
        pools['mT0'] = mT0p.tile([P, 8, QBS], BF16, name="mT0", tag="mT0")
        nc.gpsimd.dma_start(out=pools['mT0'], in_=mr[:, 0:8, 0:QBS])
        # on_act=False: these run woven into qb0 attention where ACT is the
        # exp-bound critical chain -- their epilogues go to DVE instead
        kv_fill = [(lambda hp=hp, sb=sb: k_unit(hp, sb, on_act=False))
                   for sb in (2, 3) for hp in range(HP)] + \
                  [(lambda kb=kb, nh=nh: v_unit(kb, nh, on_act=False))
                   for kb in range(8, KB) for nh in range(NH)]
        for hp in range(HP):
            attn_iter(hp, 0, kv_fill)
        for fn in kv_fill:
            fn()
        kv_fill.clear()
        kv_in.release()
        proj_ps.release()
        mT0p.release()

        # ---------- qb1 attention with qb0 epilogue woven in ----------
        mid_ow = tc.alloc_tile_pool(name="mid_ow", bufs=1, side="right")
        xres_sb = mid_ow.tile([P, QC, D], F32)
        nc.gpsimd.dma_start(out=xres_sb,
                            in_=xres.rearrange("(c p) n -> p c n", p=P))
        wo_sb = mid_ow.tile([P, DC, D], BF16)
        nc.gpsimd.dma_start(out=wo_sb, in_=wo.rearrange("(c p) n -> p c n", p=P))
        op_ps = tc.alloc_tile_pool(name="op_ps", bufs=1, space="PSUM", side="right")
        tp_ps = tc.alloc_tile_pool(name="tp_ps", bufs=1, space="PSUM", side="right")

        hpre_map = {}

        def op_half(qc, nh):
            def go():
                if qc not in hpre_map:
                    hpre_map[qc] = ln_wk.tile([P, D], F32, tag="hpre",
                                              bufs=3, name=f"hpre_{qc}")
                hpre = hpre_map[qc]
                ps = op_ps.tile([P, NHW], F32, tag="op")
                for c in range(DC):
                    nc.tensor.matmul(
                        ps, lhsT=ctxT[:, c, qc * P : (qc + 1) * P],
                        rhs=wo_sb[:, c, nh * NHW : (nh + 1) * NHW],
                        start=(c == 0), stop=(c == DC - 1),
                    )
                nc.vector.scalar_tensor_tensor(
                    out=hpre[:, nh * NHW : (nh + 1) * NHW], in0=ps,
                    scalar=1.0, in1=xres_sb[:, qc, nh * NHW : (nh + 1) * NHW],
                    op0=OP.mult, op1=OP.add,
                )
            return go

        def ln_unit(qc):
            def go():
                layer_norm(ln_wk, hpre_map.pop(qc), g1b, be1b, h_sb[:, qc, :])
            return go

        def transp_half(qc, lo):
            def go():
                for c in range(lo, lo + DC // 2):
                    tp = tp_ps.tile([P, P], BF16, tag="tp")
                    nc.tensor.transpose(tp, h_sb[:, qc, c * P : (c + 1) * P],
                                        ident)
                    nc.scalar.activation(out=hT[:, c, qc * P : (qc + 1) * P],
                                         in_=tp, func=AF.Copy)
            return go

        def outproj_unit(qc):
            def go():
                op_half(qc, 0)()
                op_half(qc, 1)()
                ln_unit(qc)()
            return go

        def transp_unit(qc):
            def go():
                for c in range(DC):
                    tp = tp_ps.tile([P, P], BF16, tag="tp")
                    nc.tensor.transpose(tp, h_sb[:, qc, c * P : (c + 1) * P],
                                        ident)
                    nc.scalar.activation(out=hT[:, c, qc * P : (qc + 1) * P],
                                         in_=tp, func=AF.Copy)
            return go

        fillers = []
        for qc in range(4):
            fillers += [op_half(qc, 0), op_half(qc, 1), ln_unit(qc)]
        for qc in range(4):
            fillers += [transp_half(qc, 0), transp_half(qc, DC // 2)]
        for hp in range(HP):
            attn_iter(hp, 1, fillers)
        for fn in pending:
            fn()
        pending.clear()
        for fn in fillers:
            fn()
        pools['nm_sb'].release()
        pools['at_sb'].release()

    attn_in.release()

    # prefetch the 9.4MB w1/w2 load so it overlaps the qc4-7 epilogue
    # instead of stalling the FFN phase start
    w12_in = tc.alloc_tile_pool(name="w12_in", bufs=1)
    w1_sb = w12_in.tile([P, DC, DF], BF16)
    # two contiguous chunk-halves on separate queues: halves the w1 wait
    # that stalls the FFN1 start ~7us (contiguous slices keep the
    # descriptor count flat, unlike a column-strided split)
    w1r = w1.rearrange("(c p) n -> p c n", p=P)
    nc.sync.dma_start(out=w1_sb[:, 0:3, :], in_=w1r[:, 0:3, :])
    nc.scalar.dma_start(out=w1_sb[:, 3:6, :], in_=w1r[:, 3:6, :])
    w2_sb = w12_in.tile([P, DFC, D], F8E4)
    nc.gpsimd.dma_start(out=w2_sb, in_=w2.rearrange("(c p) n -> p c n", p=P))

    # ====== FFN: f1^T = gelu(w1^T h^T + b1); out = LN2(f1g^T w2 + h) ====
    with tc.tile_pool(name="f1_ps", bufs=3, space="PSUM") as f1_ps, \
         tc.tile_pool(name="f2_ps", bufs=3, space="PSUM") as f2_ps, \
         tc.tile_pool(name="f1g_sb", bufs=2) as f1g_sb, \
         tc.tile_pool(name="out_sb", bufs=3) as out_sb:

        def ffn1(qb, fill=None):
            qs = slice(qb * QBS, (qb + 1) * QBS)
            # fp8: FC2 runs as DoubleRow (2 k-tiles/pass, ~1.4x); gelu
            # outputs |g|<~8 and w2~0.02 are far inside e4m3 range, and the
            # FC2-only quantization error (~1.4e-2) fits the 2e-2 tolerance
            f1g = f1g_sb.tile([P, DFC, QBS], F8E4, tag="f1g",
                              name=f"f1g_{qb}")
            for f in range(DFC):
                ps = f1_ps.tile([P, QBS], F32, tag="f1")
                for c in range(DC):
                    nc.tensor.matmul(
                        ps, lhsT=w1_sb[:, c, f * P : (f + 1) * P],
                        rhs=hT[:, c, qs], start=(c == 0), stop=(c == DC - 1),
                    )
                nc.scalar.activation(out=f1g[:, f, :], in_=ps, func=AF.Gelu,
                                     bias=b1p[:, f : f + 1])
                if fill:
                    fill.pop(0)()
            return f1g

        def ffn2(qb, f1g):
            for sq in range(QBS // P):
                qc = qb * (QBS // P) + sq
                ot = out_sb.tile([P, D], F32, tag="ot")
                for nh in range(NH):
                    ps = f2_ps.tile([P, NHW], F32, tag="f2")
                    for fp in range(DFC // 2):
                        nc.tensor.matmul(
                            ps,
                            lhsT=f1g[:, 2 * fp : 2 * fp + 2,
                                     sq * P : (sq + 1) * P],
                            rhs=w2_sb[:, 2 * fp : 2 * fp + 2,
                                      nh * NHW : (nh + 1) * NHW],
                            start=(fp == 0), stop=(fp == DFC // 2 - 1),
                            perf_mode=mybir.MatmulPerfMode.DoubleRow,
                        )
                    nc.vector.scalar_tensor_tensor(
                        out=ot[:, nh * NHW : (nh + 1) * NHW], in0=ps,
                        scalar=1.0,
                        in1=h_sb[:, qc, nh * NHW : (nh + 1) * NHW],
                        op0=OP.mult, op1=OP.add,
                    )
                nc.vector.tensor_tensor(out=ot, in0=ot, in1=b2b, op=OP.add)
                layer_norm(ln_wk, ot, g2b, be2b, ot)
                nc.sync.dma_start(out=out[qc * P : (qc + 1) * P, :], in_=ot)

        # ---- qc4-7 out-projection epilogue woven INTO FFN1(qb0): FFN1 qb0
        # only needs hT qc0-3 (built during the qb1 fillers). Each op_half
        # rides between two dense f-units so its bufs=1 PSUM ring drains
        # behind PE work instead of serializing against the DVE adds
        epi = []
        for qc in range(4, QC):
            epi += [op_half(qc, 0), op_half(qc, 1), ln_unit(qc)]
        f1g0 = ffn1(0, epi)
        for qc in range(4, QC):
            transp_unit(qc)()
        ffn2(0, f1g0)
        ffn2(1, ffn1(1))

    tp_ps.release()
    op_ps.release()
    mid_ow.release()
    mid_ctx.release()

    w12_in.release()
    ln_wk.release()
    ffn.release()


def build_program():
    nc = bacc.Bacc("TRN2", target_bir_lowering=False, debug=False,
                   enable_asserts=False, num_devices=N_CORES)
    io = {}

    def din(name, shape, dt):
        io[name] = nc.dram_tensor(name, list(shape), dt, kind="ExternalInput").ap()

    din("xT", (D, S), BF16)
    din("xqT", (D, SQ), BF16)
    din("xres", (SQ, D), F32)
    din("maskT", (S, SQ), BF16)
    din("wq", (D, D), BF16)
    din("wk", (D, D), BF16)
    din("wv", (D, D), BF16)
    din("wo", (D, D), BF16)
    din("w1", (D, DF), BF16)
    din("w2", (DF, D), F8E4)
    din("bqp", (P, HP), F32)
    din("bkp", (P, HP), F32)
    din("b1p", (P, DFC), F32)
    for n in ["g1r", "be1r", "g2r", "be2r", "b2r"]:
        din(n, (1, D), F32)
    io["out"] = nc.dram_tensor("out", [SQ, D], F32, kind="ExternalOutput").ap()

    with tile.TileContext(nc) as tc:
        with ExitStack() as ctx:
            emit(ctx, tc, io)
    nc.compile()
    return nc


_NC = None


def _get_program():
    global _NC
    if _NC is None:
        _NC = build_program()
    return _NC


def _qrows(half):
    if half == 0:
        return np.concatenate([np.arange(0, 512), np.arange(1536, 2048)])
    return np.arange(512, 1536)


def shard_inputs(inputs):
    x = np.asarray(inputs["x"], np.float32)
    mask = np.asarray(inputs["mask"], bool)
    w = {k: np.asarray(inputs[k], np.float32) for k in
         ["wq", "bq", "wk", "bk", "wv", "bv", "wo", "bo", "g1", "be1",
          "w1", "b1", "w2", "b2", "g2", "be2"]}

    base = dict(
        wq=np.ascontiguousarray(w["wq"].astype(BF)),
        wk=np.ascontiguousarray(w["wk"].astype(BF)),
        wv=np.ascontiguousarray(w["wv"].astype(BF)),
        wo=np.ascontiguousarray(w["wo"].astype(BF)),
        w1=np.ascontiguousarray(w["w1"].astype(BF)),
        w2=np.ascontiguousarray(w["w2"].astype(ml_dtypes.float8_e4m3)),
        bqp=np.ascontiguousarray(w["bq"].reshape(HP, P).T),
        bkp=np.ascontiguousarray(w["bk"].reshape(HP, P).T),
        b1p=np.ascontiguousarray(w["b1"].reshape(DFC, P).T),
        g1r=np.ascontiguousarray(w["g1"].reshape(1, D)),
        be1r=np.ascontiguousarray(w["be1"].reshape(1, D)),
        g2r=np.ascontiguousarray(w["g2"].reshape(1, D)),
        be2r=np.ascontiguousarray(w["be2"].reshape(1, D)),
        b2r=np.ascontiguousarray(w["b2"].reshape(1, D)),
    )
    # bv and bo fold into the residual: ctx@wo + bo + x with v-bias bv adds
    # a constant row bv@wo (softmax rows sum to 1)
    res_const = (w["bo"] + w["bv"] @ w["wo"]).astype(np.float32)

    in_maps = []
    for c in range(N_CORES):
        b, half = divmod(c, 2)
        qr = _qrows(half)
        xb = x[b]
        xq = xb[qr]
        m = dict(base)
        m["xT"] = np.ascontiguousarray(xb.T.astype(BF))
        m["xqT"] = np.ascontiguousarray(xq.T.astype(BF))
        m["xres"] = np.ascontiguousarray(xq + res_const[None, :])
        m["maskT"] = np.ascontiguousarray((~mask[b][qr]).T.astype(BF))
        in_maps.append(m)
    return in_maps


def gather_outputs(results):
    y = np.empty((B, S, D), np.float32)
    for c in range(N_CORES):
        b, half = divmod(c, 2)
        y[b, _qrows(half)] = results[c]["out"]
    return y


def kernel(**inputs):
    nc = _get_program()
    in_maps = shard_inputs(inputs)
    res = run_bass_kernel_spmd(nc, in_maps, list(range(N_CORES)))
    return gather_outputs(res.results)


if __name__ == "__main__":
    build_program()
    print("program built ok")



# revision 64
# speedup vs baseline: 1.0510x; 1.0294x over previous
"""Trainium2 Bass kernel for a dense transformer decoder block.

Reference computation (B=4, S=2048, D=768, H=12, DK=64, DF=3072):
    q,k,v = x@wq+bq, x@wk+bk, x@wv+bv          (per-head split, DK=64)
    attn  = softmax(mask(q k^T / 8))
    ctx   = attn @ v
    h     = LN(ctx@wo + bo + x; g1, be1)
    out   = LN(gelu_exact(h@w1 + b1)@w2 + b2 + h; g2, be2)

Sharding: pure data parallel, zero collectives. 8 cores = 4 batch elements
x 2 query groups of 1024 rows. Queries are paired so each core's two
512-row query blocks need key extents {<=8, <=16} key-blocks of 128
(block-causal skip); the exact mask is applied as data.
Core 2b+0: query rows [0:512) u [1536:2048) of batch b.
Core 2b+1: query rows [512:1536) of batch b.
Every core runs the identical SPMD program; per-core behavior differs only
through input data (sliced/transposed/cast on the host).

Schedule: attention is ACT-(exp)-bound, so independent PE work is woven
between attention iterations to keep the tensor engine dense (and its HAM
clock warm): the sb2/sb3 K,V projections run under qb0 attention, and the
qb0 out-projection + LN1 + h-transposes run under qb1 attention.
"""

from contextlib import ExitStack

import numpy as np
import ml_dtypes

import concourse.bass as bass
import concourse.tile as tile
from concourse import bacc, mybir
from concourse.bass_utils import run_bass_kernel_spmd
from concourse.masks import make_identity

F32 = mybir.dt.float32
I32 = mybir.dt.int32
BF16 = mybir.dt.bfloat16
F8E4 = mybir.dt.float8e4
AF = mybir.ActivationFunctionType
OP = mybir.AluOpType
BF = ml_dtypes.bfloat16

B, S, D, H, DK = 4, 2048, 768, 12, 64
DF = 4 * D
EPS = 1e-5
P = 128
SQ = 1024            # query rows per core
HP = H // 2          # 6 head pairs
KB = S // P          # 16 key blocks
QB = 2               # query blocks of 512 per core
QBS = 512
EXT = [8, 16]        # key-block extent per query block (block-causal skip)
DC = D // P          # 6 chunks of the model dim
DFC = DF // P        # 24 chunks of the FFN dim
QC = SQ // P         # 8 query chunks of 128
NH = 2               # 384-wide halves of D for PSUM-friendly matmul N
NHW = D // NH        # 384
SB = S // QBS        # 4 key column slabs

N_CORES = 8


def emit(ctx: ExitStack, tc: tile.TileContext, io: dict):
    nc = tc.nc

    xT, xqT, xres, maskT = io["xT"], io["xqT"], io["xres"], io["maskT"]
    wq, wk, wv, wo, w1, w2 = io["wq"], io["wk"], io["wv"], io["wo"], io["w1"], io["w2"]
    out = io["out"]

    # ---- constants ----------------------------------------------------
    const = ctx.enter_context(tc.tile_pool(name="const", bufs=1))
    ident = const.tile([P, P], BF16)
    make_identity(nc, ident)
    eps_t = const.tile([P, 1], F32)
    nc.vector.memset(eps_t, EPS)

    # PE warmup: the first input DMA bytes land ~9us in and the first
    # projection matmul issues ~22us in; without sustained PE activity the
    # HAM clock gate holds the array at 1.2GHz for the first ~3.4us of real
    # work. Junk matmuls on the on-chip identity span the DMA wait so the
    # projections start at 2.4GHz.
    warm_ps = tc.alloc_tile_pool(name="warm_ps", bufs=1, space="PSUM")
    wps = warm_ps.tile([64, 64], F32)
    for _ in range(220):
        nc.tensor.matmul(wps, lhsT=ident[:, 0:64], rhs=ident[:, 0:64],
                         start=True, stop=True)
    warm_ps.release()

    bqp = const.tile([P, HP], F32)
    nc.gpsimd.dma_start(out=bqp, in_=io["bqp"])
    bkp = const.tile([P, HP], F32)
    nc.gpsimd.dma_start(out=bkp, in_=io["bkp"])
    b1p = const.tile([P, DFC], F32)

    def brow_alloc(name):
        # [1, D] dram tensor broadcast-DMA'd across 128 partitions; tiles
        # allocated here, transfers issued after the projection inputs so
        # the first matmuls aren't queued behind 1.9MB of LN constants
        return const.tile([P, D], F32, tag=name, name=name)

    g1b, be1b, g2b, be2b, b2b = map(
        brow_alloc, ["g1r", "be1r", "g2r", "be2r", "b2r"])
    _brow_tiles = {"g1r": g1b, "be1r": be1b, "g2r": g2b, "be2r": be2b,
                   "b2r": b2b}

    def issue_ln_const_dmas():
        nc.gpsimd.dma_start(out=b1p, in_=io["b1p"])
        for name, t in _brow_tiles.items():
            a = io[name]
            src = bass.AP(tensor=a.tensor, offset=a.offset,
                          ap=[[0, P]] + list(a.ap[1:]))
            nc.gpsimd.dma_start(out=t, in_=src)

    # ---- FFN-phase tensors: left stack, below attn_in so release order
    # stays LIFO (h/hT are written during the attention epilogue fillers)
    ffn = tc.alloc_tile_pool(name="ffn", bufs=1)
    h_sb = ffn.tile([P, QC, D], BF16)     # LN1 out (residual + FFN rhs)
    hT = ffn.tile([P, DC, SQ], BF16)
    ln_wk = tc.alloc_tile_pool(name="ln_wk", bufs=1)

    # ---- attention inputs (live through attention) --------------------
    attn_in = tc.alloc_tile_pool(name="attn_in", bufs=1)
    KT = attn_in.tile([P, HP, S], BF16)            # K^T, head pairs on partitions
    Vaug = attn_in.tile([P, KB, H, DK + 1], BF16)  # V + ones column per head
    QT = attn_in.tile([P, HP, SQ], BF16)
    # only the mask quadrants that aren't structurally skipped:
    # qb0 masks key blocks 0..7, qb1 masks key blocks 8..15
    mT1 = attn_in.tile([P, 8, QBS], BF16)
    mr = maskT.rearrange("(kb p) q -> p kb q", p=P)
    nc.vector.memset(Vaug[:, :, :, DK : DK + 1], 1.0)

    # ---- post-attention inputs (right-side stack, phase-scoped) -------
    mid_ctx = tc.alloc_tile_pool(name="mid_ctx", bufs=1, side="right")
    ctxT = mid_ctx.tile([P, DC, SQ], BF16)

    kv_in = tc.alloc_tile_pool(name="kv_in", bufs=1, side="right")
    wk_sb = kv_in.tile([P, DC, D], BF16)
    wv_sb = kv_in.tile([P, DC, D], BF16)
    xT23 = kv_in.tile([P, DC, S // 2], BF16)
    xt01p = tc.alloc_tile_pool(name="xt01p", bufs=1, side="right")
    xT01 = xt01p.tile([P, DC, S // 2], BF16)
    xTr = xT.rearrange("(c p) s -> p c s", p=P)

    def xT_at(sb):
        t = xT01 if sb < 2 else xT23
        return t, (sb % 2) * QBS

    q_in = tc.alloc_tile_pool(name="q_in", bufs=1, side="right")
    wq_sb = q_in.tile([P, DC, D], BF16)
    xqT_sb = q_in.tile([P, DC, SQ], BF16)
    nc.sync.dma_start(out=wq_sb, in_=wq.rearrange("(c p) n -> p c n", p=P))
    xqr = xqT.rearrange("(c p) s -> p c s", p=P)
    for sb in range(2):
        nc.scalar.dma_start(out=xqT_sb[:, :, sb * QBS : (sb + 1) * QBS],
                            in_=xqr[:, :, sb * QBS : (sb + 1) * QBS])
    for sb in range(SB):
        dst = xT01 if sb < 2 else xT23
        nc.sync.dma_start(out=dst[:, :, (sb % 2) * QBS : (sb % 2 + 1) * QBS],
                          in_=xTr[:, :, sb * QBS : (sb + 1) * QBS])
    # wk on the gpsimd queue, wv on scalar: spreads the projection-input
    # load across three DGE queues instead of two
    for c in range(DC):
        nc.gpsimd.dma_start(out=wk_sb[:, c, :],
                            in_=wk.rearrange("(c p) n -> p c n", p=P)[:, c, :])
    for c in range(DC):
        nc.gpsimd.dma_start(out=wv_sb[:, c, :],
                            in_=wv.rearrange("(c p) n -> p c n", p=P)[:, c, :])
    # mask + LN constants: not needed until well after the projections,
    # so these transfers queue behind the weight/x loads
    issue_ln_const_dmas()
    nc.gpsimd.dma_start(out=mT1, in_=mr[:, 8:16, QBS : 2 * QBS])

    def layer_norm(wk_pool, src, gb, bb, dst):
        # LN over the free dim (768) of fp32 src [128, 768]; dst may be the
        # same tile or a bf16 target
        stats = wk_pool.tile([P, 2, 6], F32, tag="stats", bufs=3)
        for j in range(2):
            nc.vector.bn_stats(out=stats[:, j, :], in_=src[:, j * 384 : (j + 1) * 384])
        mv = wk_pool.tile([P, 2], F32, tag="mv", bufs=3)
        nc.vector.bn_aggr(out=mv, in_=stats)
        # rstd = rsqrt(var+eps) entirely on DVE (Quake seed + 3 Newton
        # steps, ~1e-7 rel err): an ACT Sqrt here would force a ~2.7us
        # activation-table swap against the attention exps / FFN gelus
        vh = wk_pool.tile([P, 1], F32, tag="vh", bufs=3)
        nc.vector.tensor_scalar(out=vh, in0=mv[:, 1:2], scalar1=EPS,
                                scalar2=-0.5, op0=OP.add, op1=OP.mult)
        seed = wk_pool.tile([P, 1], I32, tag="seed", bufs=3)
        nc.vector.tensor_scalar(out=seed, in0=mv[:, 1:2].bitcast(I32),
                                scalar1=1, scalar2=None,
                                op0=OP.logical_shift_right)
        nc.vector.tensor_scalar(out=seed, in0=seed, scalar1=-1,
                                scalar2=0x5F3759DF, op0=OP.mult, op1=OP.add)
        rstd = wk_pool.tile([P, 1], F32, tag="rstd", bufs=3)
        y = seed.bitcast(F32)
        for _ in range(3):
            w = wk_pool.tile([P, 1], F32, tag="nw", bufs=3)
            nc.vector.scalar_tensor_tensor(out=w, in0=y, scalar=y,
                                           in1=vh, op0=OP.mult, op1=OP.mult)
            nc.vector.tensor_scalar(out=w, in0=w, scalar1=1.5, scalar2=None,
                                    op0=OP.add)
            nc.vector.tensor_tensor(out=rstd, in0=y, in1=w, op=OP.mult)
            y = rstd
        nc.vector.tensor_scalar_sub(out=src, in0=src, scalar1=mv[:, 0:1])
        nc.vector.scalar_tensor_tensor(out=src, in0=src, scalar=rstd,
                                       in1=gb, op0=OP.mult, op1=OP.mult)
        nc.vector.tensor_tensor(out=dst, in0=src, in1=bb, op=OP.add)

    proj_ps = tc.alloc_tile_pool(name="proj_ps", bufs=2, space="PSUM", side="right")
    with tc.tile_pool(name="sc_ps", bufs=2, space="PSUM") as sc_ps, \
         tc.tile_pool(name="cx_ps", bufs=1, space="PSUM") as cx_ps:
        # at_sb/nm_sb/mT0 are allocated only once the q-projection inputs are
        # released -- their SBUF footprints must not overlap
        pools = {}

        # ---------- projection work units ----------
        def q_unit(hp, sb):
            ps = proj_ps.tile([P, QBS], F32, tag="proj")
            for c in range(DC):
                nc.tensor.matmul(
                    ps, lhsT=wq_sb[:, c, hp * P : (hp + 1) * P],
                    rhs=xqT_sb[:, c, sb * QBS : (sb + 1) * QBS],
                    start=(c == 0), stop=(c == DC - 1),
                )
            # bias on DVE: the ACT FIFO is clogged with DMA-descriptor and
            # semaphore instructions at startup, which left the PE stalled
            # ~8us on the proj_ps ring waiting for the first bias reads
            nc.vector.tensor_scalar_add(
                out=QT[:, hp, sb * QBS : (sb + 1) * QBS], in0=ps,
                scalar1=bqp[:, hp : hp + 1],
            )

        def k_unit(hp, sb, on_act=True):
            xt, off = xT_at(sb)
            ps = proj_ps.tile([P, QBS], F32, tag="proj")
            for c in range(DC):
                nc.tensor.matmul(
                    ps, lhsT=wk_sb[:, c, hp * P : (hp + 1) * P],
                    rhs=xt[:, c, off : off + QBS],
                    start=(c == 0), stop=(c == DC - 1),
                )
            if on_act:
                nc.scalar.activation(
                    out=KT[:, hp, sb * QBS : (sb + 1) * QBS], in_=ps,
                    func=AF.Identity, bias=bkp[:, hp : hp + 1],
                )
            else:
                # inside the attention interleave ACT is the bottleneck chain
                nc.vector.tensor_scalar_add(
                    out=KT[:, hp, sb * QBS : (sb + 1) * QBS], in0=ps,
                    scalar1=bkp[:, hp : hp + 1],
                )

        def v_unit(kb, nh, on_act=True):
            xt, off = xT_at(kb // (QBS // P))
            kb_off = off // P + kb % (QBS // P)
            ps = proj_ps.tile([P, QBS], F32, tag="proj")
            psv = ps[:, 0:NHW]
            for c in range(DC):
                nc.tensor.matmul(
                    psv, lhsT=xt[:, c, kb_off * P : (kb_off + 1) * P],
                    rhs=wv_sb[:, c, nh * NHW : (nh + 1) * NHW],
                    start=(c == 0), stop=(c == DC - 1),
                )
            if on_act:
                nc.scalar.activation(
                    out=Vaug[:, kb, nh * 6 : (nh + 1) * 6, 0:DK],
                    in_=psv.rearrange("p (h d) -> p h d", d=DK),
                    func=AF.Copy,
                )
            else:
                nc.vector.tensor_copy(
                    out=Vaug[:, kb, nh * 6 : (nh + 1) * 6, 0:DK],
                    in_=psv.rearrange("p (h d) -> p h d", d=DK),
                )

        def kv_slab(sb, on_act=True):
            for hp in range(HP):
                k_unit(hp, sb, on_act)
            for j in range(QBS // P):
                for nh in range(NH):
                    v_unit(sb * (QBS // P) + j, nh, on_act)

        # ---------- attention iteration ----------
        pending = []

        def make_norm(cxs_e, cxs_o, den2, hp, qs):
            def go():
                # one reciprocal serves both heads: its cost scales with the
                # free size, not the partition count
                rec2 = pools['nm_sb'].tile([DK + 1, QBS], F32, tag="rec2", bufs=1)
                # softmax denominators are well-conditioned (>=1, <~4e3):
                # the ~5x faster 18-bit approx is far inside tolerance
                nc.vector.reciprocal_approx_fast(rec2, den2)
                # partition_broadcast replicates the tile's physical partition
                # 0, so the head-odd reciprocal must move to its own base-0
                # tile first
                rec_o = pools['nm_sb'].tile([1, QBS], F32, tag="rec_o", bufs=1)
                nc.vector.tensor_copy(out=rec_o, in_=rec2[DK : DK + 1, :])
                for i, (cxs, pb) in enumerate(((cxs_e, 0), (cxs_o, DK))):
                    src_r = rec2[0:1, :] if i == 0 else rec_o[0:1, :]
                    den_b = pools['nm_sb'].tile([DK, QBS], F32, tag="den_b", bufs=1)
                    nc.gpsimd.partition_broadcast(den_b, src_r)
                    nc.vector.tensor_tensor(
                        out=ctxT[pb : pb + DK, hp, qs], in0=cxs[0:DK, :],
                        in1=den_b, op=OP.mult,
                    )
            return go

        def attn_iter(hp, qb, fill=None):
            ext = EXT[qb]
            qs = slice(qb * QBS, (qb + 1) * QBS)
            cx_e = cx_ps.tile([DK + 1, QBS], F32, tag="cx_e")
            cx_o = cx_ps.tile([DK + 1, QBS], F32, tag="cx_o")
            def front(gb):
                # scores + exp (+ mask) for key-block pair gb
                pt = pools['at_sb'].tile([P, 2, 2, QBS], BF16, tag="pt")
                for gi in range(2):
                    g = gb + gi
                    ks = slice(g * P, (g + 1) * P)
                    sc = sc_ps.tile([P, 2, QBS], F32, tag="sc")
                    # the two heads of a pair hit disjoint PE row groups and
                    # run concurrently in the array
                    nc.tensor.matmul(sc[:, 0, :], lhsT=KT[0:DK, hp, ks],
                                     rhs=QT[0:DK, hp, qs], start=True, stop=True)
                    nc.tensor.matmul(sc[:, 1, :], lhsT=KT[DK:P, hp, ks],
                                     rhs=QT[DK:P, hp, qs], start=True, stop=True)
                    nc.scalar.activation(out=pt[:, :, gi, :], in_=sc,
                                         func=AF.Exp, scale=1.0 / 8.0)
                # qb0 masks kb 0..7 via mT0; qb1 masks only kb 8..15 (below
                # the diagonal for every core) via mT1
                if qb == 0 or gb >= 8:
                    mq = pools['mT0'][:, gb : gb + 2, :] if qb == 0 else \
                        mT1[:, gb - 8 : gb - 6, :]
                    # one multiply for both heads: the mask AP broadcasts
                    # over the head dim with a 0-stride
                    mqb = bass.AP(tensor=mq.tensor, offset=mq.offset,
                                  ap=[mq.ap[0], [0, 2]] + list(mq.ap[1:]))
                    nc.vector.tensor_tensor(out=pt, in0=pt, in1=mqb,
                                            op=OP.mult)
                return pt

            def back(gb, pt):
                for gi in range(2):
                    g = gb + gi
                    nc.tensor.matmul(cx_e, lhsT=Vaug[:, g, 2 * hp, :],
                                     rhs=pt[:, 0, gi, :],
                                     start=(g == 0), stop=(g == ext - 1))
                    nc.tensor.matmul(cx_o, lhsT=Vaug[:, g, 2 * hp + 1, :],
                                     rhs=pt[:, 1, gi, :],
                                     start=(g == 0), stop=(g == ext - 1))
                if gb == 2 and pending:
                    pending.pop()()
                if fill and (qb == 0 or gb % 4 == 2):
                    fill.pop(0)()

            # one-pair software pipeline: pair t+1's score matmuls issue
            # BEFORE pair t's ctx matmuls/fillers, so the exp chain on ACT
            # (the qb1 bottleneck) never starves behind PE queue order --
            # S(t,a) S(t,b) C(t-1) takes ~1.7us, just past exp(t,a)'s 1.57us
            prev = None
            for gb in range(0, ext, 2):
                pt = front(gb)
                if prev is not None:
                    back(*prev)
                prev = (gb, pt)
            back(*prev)
            # stage ctx to SBUF immediately: frees the PSUM bank within one
            # DVE copy so the cx pool gets away with a single buffer
            cxs_e = pools['nm_sb'].tile([DK + 1, QBS], F32, tag="cxs_e")
            nc.vector.tensor_copy(out=cxs_e, in_=cx_e)
            cxs_o = pools['nm_sb'].tile([DK + 1, QBS], F32, tag="cxs_o")
            nc.vector.tensor_copy(out=cxs_o, in_=cx_o)
            den2 = pools['nm_sb'].tile([DK + 1, QBS], F32, tag="den2")
            nc.vector.memset(den2, 1.0)
            nc.vector.tensor_copy(out=den2[0:1, :], in_=cx_e[DK : DK + 1, :])
            nc.vector.tensor_copy(out=den2[DK : DK + 1, :], in_=cx_o[DK : DK + 1, :])
            pending.append(make_norm(cxs_e, cxs_o, den2, hp, qs))

        # ---------- schedule: projections + qb0 attention ----------
        for hp in range(HP):
            q_unit(hp, 0)
        kv_slab(0, on_act=False)
        kv_slab(1, on_act=False)
        for hp in range(HP):
            q_unit(hp, 1)
        q_in.release()
        xt01p.release()
        pools['at_sb'] = tc.alloc_tile_pool(name="at_sb", bufs=3)
        pools['nm_sb'] = tc.alloc_tile_pool(name="nm_sb", bufs=2)
        mT0p = tc.alloc_tile_pool(name="mT0p", bufs=1)
        pools['mT0'] = mT0p.tile([P, 8, QBS], BF16, name="mT0", tag="mT0")
        nc.gpsimd.dma_start(out=pools['mT0'], in_=mr[:, 0:8, 0:QBS])
        # on_act=False: these run woven into qb0 attention where ACT is the
        # exp-bound critical chain -- their epilogues go to DVE instead
        kv_fill = [(lambda hp=hp, sb=sb: k_unit(hp, sb, on_act=False))
                   for sb in (2, 3) for hp in range(HP)] + \
                  [(lambda kb=kb, nh=nh: v_unit(kb, nh, on_act=False))
                   for kb in range(8, KB) for nh in range(NH)]
        for hp in range(HP):
            attn_iter(hp, 0, kv_fill)
        for fn in kv_fill:
            fn()
        kv_fill.clear()
        kv_in.release()
        proj_ps.release()
        mT0p.release()

        # ---------- qb1 attention with qb0 epilogue woven in ----------
        mid_ow = tc.alloc_tile_pool(name="mid_ow", bufs=1, side="right")
        xres_sb = mid_ow.tile([P, QC, D], F32)
        nc.gpsimd.dma_start(out=xres_sb,
                            in_=xres.rearrange("(c p) n -> p c n", p=P))
        wo_sb = mid_ow.tile([P, DC, D], BF16)
        nc.gpsimd.dma_start(out=wo_sb, in_=wo.rearrange("(c p) n -> p c n", p=P))
        op_ps = tc.alloc_tile_pool(name="op_ps", bufs=1, space="PSUM", side="right")
        tp_ps = tc.alloc_tile_pool(name="tp_ps", bufs=1, space="PSUM", side="right")

        hpre_map = {}

        def op_half(qc, nh):
            def go():
                if qc not in hpre_map:
                    hpre_map[qc] = ln_wk.tile([P, D], F32, tag="hpre",
                                              bufs=3, name=f"hpre_{qc}")
                hpre = hpre_map[qc]
                ps = op_ps.tile([P, NHW], F32, tag="op")
                for c in range(DC):
                    nc.tensor.matmul(
                        ps, lhsT=ctxT[:, c, qc * P : (qc + 1) * P],
                        rhs=wo_sb[:, c, nh * NHW : (nh + 1) * NHW],
                        start=(c == 0), stop=(c == DC - 1),
                    )
                nc.vector.scalar_tensor_tensor(
                    out=hpre[:, nh * NHW : (nh + 1) * NHW], in0=ps,
                    scalar=1.0, in1=xres_sb[:, qc, nh * NHW : (nh + 1) * NHW],
                    op0=OP.mult, op1=OP.add,
                )
            return go

        def ln_unit(qc):
            def go():
                layer_norm(ln_wk, hpre_map.pop(qc), g1b, be1b, h_sb[:, qc, :])
            return go

        def transp_half(qc, lo):
            def go():
                for c in range(lo, lo + DC // 2):
                    tp = tp_ps.tile([P, P], BF16, tag="tp")
                    nc.tensor.transpose(tp, h_sb[:, qc, c * P : (c + 1) * P],
                                        ident)
                    nc.scalar.activation(out=hT[:, c, qc * P : (qc + 1) * P],
                                         in_=tp, func=AF.Copy)
            return go

        def outproj_unit(qc):
            def go():
                op_half(qc, 0)()
                op_half(qc, 1)()
                ln_unit(qc)()
            return go

        def transp_unit(qc):
            def go():
                for c in range(DC):
                    tp = tp_ps.tile([P, P], BF16, tag="tp")
                    nc.tensor.transpose(tp, h_sb[:, qc, c * P : (c + 1) * P],
                                        ident)
                    nc.scalar.activation(out=hT[:, c, qc * P : (qc + 1) * P],
                                         in_=tp, func=AF.Copy)
            return go

        fillers = []
        for qc in range(4):
            fillers += [op_half(qc, 0), op_half(qc, 1), ln_unit(qc)]
        for qc in range(4):
            fillers += [transp_half(qc, 0), transp_half(qc, DC // 2)]
        for hp in range(HP):
            attn_iter(hp, 1, fillers)
        for fn in pending:
            fn()
        pending.clear()
        for fn in fillers:
            fn()
        pools['nm_sb'].release()
        pools['at_sb'].release()

    attn_in.release()

    # prefetch the 9.4MB w1/w2 load so it overlaps the qc4-7 epilogue
    # instead of stalling the FFN phase start
    w12_in = tc.alloc_tile_pool(name="w12_in", bufs=1)
    w1_sb = w12_in.tile([P, DC, DF], BF16)
    # two contiguous chunk-halves on separate queues: halves the w1 wait
    # that stalls the FFN1 start ~7us
    w1r = w1.rearrange("(c p) n -> p c n", p=P)
    nc.sync.dma_start(out=w1_sb[:, 0:3, :], in_=w1r[:, 0:3, :])
    nc.scalar.dma_start(out=w1_sb[:, 3:6, :], in_=w1r[:, 3:6, :])
    w2_sb = w12_in.tile([P, DFC, D], F8E4)
    nc.scalar.dma_start(out=w2_sb, in_=w2.rearrange("(c p) n -> p c n", p=P))

    # ====== FFN: f1^T = gelu(w1^T h^T + b1); out = LN2(f1g^T w2 + h) ====
    with tc.tile_pool(name="f1_ps", bufs=3, space="PSUM") as f1_ps, \
         tc.tile_pool(name="f2_ps", bufs=3, space="PSUM") as f2_ps, \
         tc.tile_pool(name="f1g_sb", bufs=2) as f1g_sb, \
         tc.tile_pool(name="out_sb", bufs=3) as out_sb:

        def ffn1(qb, fill=None):
            qs = slice(qb * QBS, (qb + 1) * QBS)
            # fp8: FC2 runs as DoubleRow (2 k-tiles/pass, ~1.4x); gelu
            # outputs |g|<~8 and w2~0.02 are far inside e4m3 range, and the
            # FC2-only quantization error (~1.4e-2) fits the 2e-2 tolerance
            f1g = f1g_sb.tile([P, DFC, QBS], F8E4, tag="f1g",
                              name=f"f1g_{qb}")
            for f in range(DFC):
                ps = f1_ps.tile([P, QBS], F32, tag="f1")
                for c in range(DC):
                    nc.tensor.matmul(
                        ps, lhsT=w1_sb[:, c, f * P : (f + 1) * P],
                        rhs=hT[:, c, qs], start=(c == 0), stop=(c == DC - 1),
                    )
                nc.scalar.activation(out=f1g[:, f, :], in_=ps, func=AF.Gelu,
                                     bias=b1p[:, f : f + 1])
                if fill:
                    fill.pop(0)()
            return f1g

        def ffn2(qb, f1g):
            for sq in range(QBS // P):
                qc = qb * (QBS // P) + sq
                ot = out_sb.tile([P, D], F32, tag="ot")
                for nh in range(NH):
                    ps = f2_ps.tile([P, NHW], F32, tag="f2")
                    for fp in range(DFC // 2):
                        nc.tensor.matmul(
                            ps,
                            lhsT=f1g[:, 2 * fp : 2 * fp + 2,
                                     sq * P : (sq + 1) * P],
                            rhs=w2_sb[:, 2 * fp : 2 * fp + 2,
                                      nh * NHW : (nh + 1) * NHW],
                            start=(fp == 0), stop=(fp == DFC // 2 - 1),
                            perf_mode=mybir.MatmulPerfMode.DoubleRow,
                        )
                    nc.vector.scalar_tensor_tensor(
                        out=ot[:, nh * NHW : (nh + 1) * NHW], in0=ps,
                        scalar=1.0,
                        in1=h_sb[:, qc, nh * NHW : (nh + 1) * NHW],
                        op0=OP.mult, op1=OP.add,
                    )
                nc.vector.tensor_tensor(out=ot, in0=ot, in1=b2b, op=OP.add)
                layer_norm(ln_wk, ot, g2b, be2b, ot)
                nc.sync.dma_start(out=out[qc * P : (qc + 1) * P, :], in_=ot)

        # ---- qc4-7 out-projection epilogue woven INTO FFN1(qb0): FFN1 qb0
        # only needs hT qc0-3 (built during the qb1 fillers). Each op_half
        # rides between two dense f-units so its bufs=1 PSUM ring drains
        # behind PE work instead of serializing against the DVE adds
        epi = []
        for qc in range(4, QC):
            epi += [op_half(qc, 0), op_half(qc, 1), ln_unit(qc)]
        f1g0 = ffn1(0, epi)
        for qc in range(4, QC):
            transp_unit(qc)()
        ffn2(0, f1g0)
        ffn2(1, ffn1(1))

    tp_ps.release()
    op_ps.release()
    mid_ow.release()
    mid_ctx.release()

    w12_in.release()
    ln_wk.release()
    ffn.release()


def build_program():
    nc = bacc.Bacc("TRN2", target_bir_lowering=False, debug=False,
                   enable_asserts=False, num_devices=N_CORES)
    io = {}

    def din(name, shape, dt):
        io[name] = nc.dram_tensor(name, list(shape), dt, kind="ExternalInput").ap()

    din("xT", (D, S), BF16)
    din("xqT", (D, SQ), BF16)
    din("xres", (SQ, D), F32)
    din("maskT", (S, SQ), BF16)
    din("wq", (D, D), BF16)
    din("wk", (D, D), BF16)
    din("wv", (D, D), BF16)
    din("wo", (D, D), BF16)
    din("w1", (D, DF), BF16)
    din("w2", (DF, D), F8E4)
    din("bqp", (P, HP), F32)
    din("bkp", (P, HP), F32)
    din("b1p", (P, DFC), F32)
    for n in ["g1r", "be1r", "g2r", "be2r", "b2r"]:
        din(n, (1, D), F32)
    io["out"] = nc.dram_tensor("out", [SQ, D], F32, kind="ExternalOutput").ap()

    with tile.TileContext(nc) as tc:
        with ExitStack() as ctx:
            emit(ctx, tc, io)
    nc.compile()
    return nc


_NC = None


def _get_program():
    global _NC
    if _NC is None:
        _NC = build_program()
    return _NC


def _qrows(half):
    if half == 0:
        return np.concatenate([np.arange(0, 512), np.arange(1536, 2048)])
    return np.arange(512, 1536)


def shard_inputs(inputs):
    x = np.asarray(inputs["x"], np.float32)
    mask = np.asarray(inputs["mask"], bool)
    w = {k: np.asarray(inputs[k], np.float32) for k in
         ["wq", "bq", "wk", "bk", "wv", "bv", "wo", "bo", "g1", "be1",
          "w1", "b1", "w2", "b2", "g2", "be2"]}

    base = dict(
        wq=np.ascontiguousarray(w["wq"].astype(BF)),
        wk=np.ascontiguousarray(w["wk"].astype(BF)),
        wv=np.ascontiguousarray(w["wv"].astype(BF)),
        wo=np.ascontiguousarray(w["wo"].astype(BF)),
        w1=np.ascontiguousarray(w["w1"].astype(BF)),
        w2=np.ascontiguousarray(w["w2"].astype(ml_dtypes.float8_e4m3)),
        bqp=np.ascontiguousarray(w["bq"].reshape(HP, P).T),
        bkp=np.ascontiguousarray(w["bk"].reshape(HP, P).T),
        b1p=np.ascontiguousarray(w["b1"].reshape(DFC, P).T),
        g1r=np.ascontiguousarray(w["g1"].reshape(1, D)),
        be1r=np.ascontiguousarray(w["be1"].reshape(1, D)),
        g2r=np.ascontiguousarray(w["g2"].reshape(1, D)),
        be2r=np.ascontiguousarray(w["be2"].reshape(1, D)),
        b2r=np.ascontiguousarray(w["b2"].reshape(1, D)),
    )
    # bv and bo fold into the residual: ctx@wo + bo + x with v-bias bv adds
    # a constant row bv@wo (softmax rows sum to 1)
    res_const = (w["bo"] + w["bv"] @ w["wo"]).astype(np.float32)

    in_maps = []
    for c in range(N_CORES):
        b, half = divmod(c, 2)
        qr = _qrows(half)
        xb = x[b]
        xq = xb[qr]
        m = dict(base)
        m["xT"] = np.ascontiguousarray(xb.T.astype(BF))
        m["xqT"] = np.ascontiguousarray(xq.T.astype(BF))
        m["xres"] = np.ascontiguousarray(xq + res_const[None, :])
        m["maskT"] = np.ascontiguousarray((~mask[b][qr]).T.astype(BF))
        in_maps.append(m)
    return in_maps


def gather_outputs(results):
    y = np.empty((B, S, D), np.float32)
    for c in range(N_CORES):
        b, half = divmod(c, 2)
        y[b, _qrows(half)] = results[c]["out"]
    return y


def kernel(**inputs):
    nc = _get_program()
    in_maps = shard_inputs(inputs)
    res = run_bass_kernel_spmd(nc, in_maps, list(range(N_CORES)))
    return gather_outputs(res.results)


if __name__ == "__main__":
    build_program()
    print("program built ok")



# revision 65
# speedup vs baseline: 1.0586x; 1.0072x over previous
"""Trainium2 Bass kernel for a dense transformer decoder block.

Reference computation (B=4, S=2048, D=768, H=12, DK=64, DF=3072):
    q,k,v = x@wq+bq, x@wk+bk, x@wv+bv          (per-head split, DK=64)
    attn  = softmax(mask(q k^T / 8))
    ctx   = attn @ v
    h     = LN(ctx@wo + bo + x; g1, be1)
    out   = LN(gelu_exact(h@w1 + b1)@w2 + b2 + h; g2, be2)

Sharding: pure data parallel, zero collectives. 8 cores = 4 batch elements
x 2 query groups of 1024 rows. Queries are paired so each core's two
512-row query blocks need key extents {<=8, <=16} key-blocks of 128
(block-causal skip); the exact mask is applied as data.
Core 2b+0: query rows [0:512) u [1536:2048) of batch b.
Core 2b+1: query rows [512:1536) of batch b.
Every core runs the identical SPMD program; per-core behavior differs only
through input data (sliced/transposed/cast on the host).

Schedule: attention is ACT-(exp)-bound, so independent PE work is woven
between attention iterations to keep the tensor engine dense (and its HAM
clock warm): the sb2/sb3 K,V projections run under qb0 attention, and the
qb0 out-projection + LN1 + h-transposes run under qb1 attention.
"""

from contextlib import ExitStack

import numpy as np
import ml_dtypes

import concourse.bass as bass
import concourse.tile as tile
from concourse import bacc, mybir
from concourse.bass_utils import run_bass_kernel_spmd
from concourse.masks import make_identity

F32 = mybir.dt.float32
I32 = mybir.dt.int32
BF16 = mybir.dt.bfloat16
F8E4 = mybir.dt.float8e4
AF = mybir.ActivationFunctionType
OP = mybir.AluOpType
BF = ml_dtypes.bfloat16

B, S, D, H, DK = 4, 2048, 768, 12, 64
DF = 4 * D
EPS = 1e-5
P = 128
SQ = 1024            # query rows per core
HP = H // 2          # 6 head pairs
KB = S // P          # 16 key blocks
QB = 2               # query blocks of 512 per core
QBS = 512
EXT = [8, 16]        # key-block extent per query block (block-causal skip)
DC = D // P          # 6 chunks of the model dim
DFC = DF // P        # 24 chunks of the FFN dim
QC = SQ // P         # 8 query chunks of 128
NH = 2               # 384-wide halves of D for PSUM-friendly matmul N
NHW = D // NH        # 384
SB = S // QBS        # 4 key column slabs

N_CORES = 8


def emit(ctx: ExitStack, tc: tile.TileContext, io: dict):
    nc = tc.nc

    xT, xqT, xres, maskT = io["xT"], io["xqT"], io["xres"], io["maskT"]
    wq, wk, wv, wo, w1, w2 = io["wq"], io["wk"], io["wv"], io["wo"], io["w1"], io["w2"]
    out = io["out"]

    # ---- constants ----------------------------------------------------
    const = ctx.enter_context(tc.tile_pool(name="const", bufs=1))
    ident = const.tile([P, P], BF16)
    make_identity(nc, ident)
    eps_t = const.tile([P, 1], F32)
    nc.vector.memset(eps_t, EPS)

    # PE warmup: the first input DMA bytes land ~9us in and the first
    # projection matmul issues ~22us in; without sustained PE activity the
    # HAM clock gate holds the array at 1.2GHz for the first ~3.4us of real
    # work. Junk matmuls on the on-chip identity span the DMA wait so the
    # projections start at 2.4GHz.
    warm_ps = tc.alloc_tile_pool(name="warm_ps", bufs=1, space="PSUM")
    wps = warm_ps.tile([64, 64], F32)
    for _ in range(220):
        nc.tensor.matmul(wps, lhsT=ident[:, 0:64], rhs=ident[:, 0:64],
                         start=True, stop=True)
    warm_ps.release()

    bqp = const.tile([P, HP], F32)
    nc.gpsimd.dma_start(out=bqp, in_=io["bqp"])
    bkp = const.tile([P, HP], F32)
    nc.gpsimd.dma_start(out=bkp, in_=io["bkp"])
    b1p = const.tile([P, DFC], F32)

    def brow_alloc(name):
        # [1, D] dram tensor broadcast-DMA'd across 128 partitions; tiles
        # allocated here, transfers issued after the projection inputs so
        # the first matmuls aren't queued behind 1.9MB of LN constants
        return const.tile([P, D], F32, tag=name, name=name)

    g1b, be1b, g2b, be2b, b2b = map(
        brow_alloc, ["g1r", "be1r", "g2r", "be2r", "b2r"])
    _brow_tiles = {"g1r": g1b, "be1r": be1b, "g2r": g2b, "be2r": be2b,
                   "b2r": b2b}

    def issue_ln_const_dmas():
        nc.gpsimd.dma_start(out=b1p, in_=io["b1p"])
        for name, t in _brow_tiles.items():
            a = io[name]
            src = bass.AP(tensor=a.tensor, offset=a.offset,
                          ap=[[0, P]] + list(a.ap[1:]))
            nc.gpsimd.dma_start(out=t, in_=src)

    # ---- FFN-phase tensors: left stack, below attn_in so release order
    # stays LIFO (h/hT are written during the attention epilogue fillers)
    ffn = tc.alloc_tile_pool(name="ffn", bufs=1)
    h_sb = ffn.tile([P, QC, D], BF16)     # LN1 out (residual + FFN rhs)
    hT = ffn.tile([P, DC, SQ], BF16)
    ln_wk = tc.alloc_tile_pool(name="ln_wk", bufs=1)

    # ---- attention inputs (live through attention) --------------------
    attn_in = tc.alloc_tile_pool(name="attn_in", bufs=1)
    KT = attn_in.tile([P, HP, S], BF16)            # K^T, head pairs on partitions
    Vaug = attn_in.tile([P, KB, H, DK + 1], BF16)  # V + ones column per head
    QT = attn_in.tile([P, HP, SQ], BF16)
    # only the mask quadrants that aren't structurally skipped:
    # qb0 masks key blocks 0..7, qb1 masks key blocks 8..15
    mT1 = attn_in.tile([P, 8, QBS], BF16)
    mr = maskT.rearrange("(kb p) q -> p kb q", p=P)
    nc.vector.memset(Vaug[:, :, :, DK : DK + 1], 1.0)

    # ---- post-attention inputs (right-side stack, phase-scoped) -------
    mid_ctx = tc.alloc_tile_pool(name="mid_ctx", bufs=1, side="right")
    ctxT = mid_ctx.tile([P, DC, SQ], BF16)

    kv_in = tc.alloc_tile_pool(name="kv_in", bufs=1, side="right")
    wk_sb = kv_in.tile([P, DC, D], BF16)
    wv_sb = kv_in.tile([P, DC, D], BF16)
    xT23 = kv_in.tile([P, DC, S // 2], BF16)
    xt01p = tc.alloc_tile_pool(name="xt01p", bufs=1, side="right")
    xT01 = xt01p.tile([P, DC, S // 2], BF16)
    xTr = xT.rearrange("(c p) s -> p c s", p=P)

    def xT_at(sb):
        t = xT01 if sb < 2 else xT23
        return t, (sb % 2) * QBS

    q_in = tc.alloc_tile_pool(name="q_in", bufs=1, side="right")
    wq_sb = q_in.tile([P, DC, D], BF16)
    xqT_sb = q_in.tile([P, DC, SQ], BF16)
    nc.sync.dma_start(out=wq_sb, in_=wq.rearrange("(c p) n -> p c n", p=P))
    xqr = xqT.rearrange("(c p) s -> p c s", p=P)
    for sb in range(2):
        nc.scalar.dma_start(out=xqT_sb[:, :, sb * QBS : (sb + 1) * QBS],
                            in_=xqr[:, :, sb * QBS : (sb + 1) * QBS])
    for sb in range(SB):
        dst = xT01 if sb < 2 else xT23
        nc.sync.dma_start(out=dst[:, :, (sb % 2) * QBS : (sb % 2 + 1) * QBS],
                          in_=xTr[:, :, sb * QBS : (sb + 1) * QBS])
    # wk on the gpsimd queue, wv on scalar: spreads the projection-input
    # load across three DGE queues instead of two
    for c in range(DC):
        nc.gpsimd.dma_start(out=wk_sb[:, c, :],
                            in_=wk.rearrange("(c p) n -> p c n", p=P)[:, c, :])
    for c in range(DC):
        nc.gpsimd.dma_start(out=wv_sb[:, c, :],
                            in_=wv.rearrange("(c p) n -> p c n", p=P)[:, c, :])
    # mask + LN constants: not needed until well after the projections,
    # so these transfers queue behind the weight/x loads
    issue_ln_const_dmas()
    nc.gpsimd.dma_start(out=mT1, in_=mr[:, 8:16, QBS : 2 * QBS])

    def layer_norm(wk_pool, src, gb, bb, dst):
        # LN over the free dim (768) of fp32 src [128, 768]; dst may be the
        # same tile or a bf16 target
        stats = wk_pool.tile([P, 2, 6], F32, tag="stats", bufs=3)
        for j in range(2):
            nc.vector.bn_stats(out=stats[:, j, :], in_=src[:, j * 384 : (j + 1) * 384])
        mv = wk_pool.tile([P, 2], F32, tag="mv", bufs=3)
        nc.vector.bn_aggr(out=mv, in_=stats)
        # rstd = rsqrt(var+eps) entirely on DVE (Quake seed + 3 Newton
        # steps, ~1e-7 rel err): an ACT Sqrt here would force a ~2.7us
        # activation-table swap against the attention exps / FFN gelus
        vh = wk_pool.tile([P, 1], F32, tag="vh", bufs=3)
        nc.vector.tensor_scalar(out=vh, in0=mv[:, 1:2], scalar1=EPS,
                                scalar2=-0.5, op0=OP.add, op1=OP.mult)
        seed = wk_pool.tile([P, 1], I32, tag="seed", bufs=3)
        nc.vector.tensor_scalar(out=seed, in0=mv[:, 1:2].bitcast(I32),
                                scalar1=1, scalar2=None,
                                op0=OP.logical_shift_right)
        nc.vector.tensor_scalar(out=seed, in0=seed, scalar1=-1,
                                scalar2=0x5F3759DF, op0=OP.mult, op1=OP.add)
        rstd = wk_pool.tile([P, 1], F32, tag="rstd", bufs=3)
        y = seed.bitcast(F32)
        for _ in range(3):
            w = wk_pool.tile([P, 1], F32, tag="nw", bufs=3)
            nc.vector.scalar_tensor_tensor(out=w, in0=y, scalar=y,
                                           in1=vh, op0=OP.mult, op1=OP.mult)
            nc.vector.tensor_scalar(out=w, in0=w, scalar1=1.5, scalar2=None,
                                    op0=OP.add)
            nc.vector.tensor_tensor(out=rstd, in0=y, in1=w, op=OP.mult)
            y = rstd
        nc.vector.tensor_scalar_sub(out=src, in0=src, scalar1=mv[:, 0:1])
        nc.vector.scalar_tensor_tensor(out=src, in0=src, scalar=rstd,
                                       in1=gb, op0=OP.mult, op1=OP.mult)
        nc.vector.tensor_tensor(out=dst, in0=src, in1=bb, op=OP.add)

    proj_ps = tc.alloc_tile_pool(name="proj_ps", bufs=2, space="PSUM", side="right")
    with tc.tile_pool(name="sc_ps", bufs=2, space="PSUM") as sc_ps, \
         tc.tile_pool(name="cx_ps", bufs=1, space="PSUM") as cx_ps:
        # at_sb/nm_sb/mT0 are allocated only once the q-projection inputs are
        # released -- their SBUF footprints must not overlap
        pools = {}

        # ---------- projection work units ----------
        def q_unit(hp, sb):
            ps = proj_ps.tile([P, QBS], F32, tag="proj")
            for c in range(DC):
                nc.tensor.matmul(
                    ps, lhsT=wq_sb[:, c, hp * P : (hp + 1) * P],
                    rhs=xqT_sb[:, c, sb * QBS : (sb + 1) * QBS],
                    start=(c == 0), stop=(c == DC - 1),
                )
            # bias on DVE: the ACT FIFO is clogged with DMA-descriptor and
            # semaphore instructions at startup, which left the PE stalled
            # ~8us on the proj_ps ring waiting for the first bias reads
            nc.vector.tensor_scalar_add(
                out=QT[:, hp, sb * QBS : (sb + 1) * QBS], in0=ps,
                scalar1=bqp[:, hp : hp + 1],
            )

        def k_unit(hp, sb, on_act=True):
            xt, off = xT_at(sb)
            ps = proj_ps.tile([P, QBS], F32, tag="proj")
            for c in range(DC):
                nc.tensor.matmul(
                    ps, lhsT=wk_sb[:, c, hp * P : (hp + 1) * P],
                    rhs=xt[:, c, off : off + QBS],
                    start=(c == 0), stop=(c == DC - 1),
                )
            if on_act:
                nc.scalar.activation(
                    out=KT[:, hp, sb * QBS : (sb + 1) * QBS], in_=ps,
                    func=AF.Identity, bias=bkp[:, hp : hp + 1],
                )
            else:
                # inside the attention interleave ACT is the bottleneck chain
                nc.vector.tensor_scalar_add(
                    out=KT[:, hp, sb * QBS : (sb + 1) * QBS], in0=ps,
                    scalar1=bkp[:, hp : hp + 1],
                )

        def v_unit(kb, nh, on_act=True):
            xt, off = xT_at(kb // (QBS // P))
            kb_off = off // P + kb % (QBS // P)
            ps = proj_ps.tile([P, QBS], F32, tag="proj")
            psv = ps[:, 0:NHW]
            for c in range(DC):
                nc.tensor.matmul(
                    psv, lhsT=xt[:, c, kb_off * P : (kb_off + 1) * P],
                    rhs=wv_sb[:, c, nh * NHW : (nh + 1) * NHW],
                    start=(c == 0), stop=(c == DC - 1),
                )
            if on_act:
                nc.scalar.activation(
                    out=Vaug[:, kb, nh * 6 : (nh + 1) * 6, 0:DK],
                    in_=psv.rearrange("p (h d) -> p h d", d=DK),
                    func=AF.Copy,
                )
            else:
                nc.vector.tensor_copy(
                    out=Vaug[:, kb, nh * 6 : (nh + 1) * 6, 0:DK],
                    in_=psv.rearrange("p (h d) -> p h d", d=DK),
                )

        def kv_slab(sb, on_act=True):
            for hp in range(HP):
                k_unit(hp, sb, on_act)
            for j in range(QBS // P):
                for nh in range(NH):
                    v_unit(sb * (QBS // P) + j, nh, on_act)

        # ---------- attention iteration ----------
        pending = []

        def make_norm(cxs_e, cxs_o, den2, hp, qs):
            def go():
                # one reciprocal serves both heads: its cost scales with the
                # free size, not the partition count
                rec2 = pools['nm_sb'].tile([DK + 1, QBS], F32, tag="rec2", bufs=1)
                # softmax denominators are well-conditioned (>=1, <~4e3):
                # the ~5x faster 18-bit approx is far inside tolerance
                nc.vector.reciprocal_approx_fast(rec2, den2)
                # partition_broadcast replicates the tile's physical partition
                # 0, so the head-odd reciprocal must move to its own base-0
                # tile first
                rec_o = pools['nm_sb'].tile([1, QBS], F32, tag="rec_o", bufs=1)
                nc.vector.tensor_copy(out=rec_o, in_=rec2[DK : DK + 1, :])
                for i, (cxs, pb) in enumerate(((cxs_e, 0), (cxs_o, DK))):
                    src_r = rec2[0:1, :] if i == 0 else rec_o[0:1, :]
                    den_b = pools['nm_sb'].tile([DK, QBS], F32, tag="den_b", bufs=1)
                    nc.gpsimd.partition_broadcast(den_b, src_r)
                    nc.vector.tensor_tensor(
                        out=ctxT[pb : pb + DK, hp, qs], in0=cxs[0:DK, :],
                        in1=den_b, op=OP.mult,
                    )
            return go

        def attn_iter(hp, qb, fill=None):
            ext = EXT[qb]
            qs = slice(qb * QBS, (qb + 1) * QBS)
            cx_e = cx_ps.tile([DK + 1, QBS], F32, tag="cx_e")
            cx_o = cx_ps.tile([DK + 1, QBS], F32, tag="cx_o")
            def front(gb):
                # scores + exp (+ mask) for key-block pair gb
                pt = pools['at_sb'].tile([P, 2, 2, QBS], BF16, tag="pt")
                for gi in range(2):
                    g = gb + gi
                    ks = slice(g * P, (g + 1) * P)
                    sc = sc_ps.tile([P, 2, QBS], F32, tag="sc")
                    # the two heads of a pair hit disjoint PE row groups and
                    # run concurrently in the array
                    nc.tensor.matmul(sc[:, 0, :], lhsT=KT[0:DK, hp, ks],
                                     rhs=QT[0:DK, hp, qs], start=True, stop=True)
                    nc.tensor.matmul(sc[:, 1, :], lhsT=KT[DK:P, hp, ks],
                                     rhs=QT[DK:P, hp, qs], start=True, stop=True)
                    nc.scalar.activation(out=pt[:, :, gi, :], in_=sc,
                                         func=AF.Exp, scale=1.0 / 8.0)
                # qb0 masks kb 0..7 via mT0; qb1 masks only kb 8..15 (below
                # the diagonal for every core) via mT1
                if qb == 0 or gb >= 8:
                    mq = pools['mT0'][:, gb : gb + 2, :] if qb == 0 else \
                        mT1[:, gb - 8 : gb - 6, :]
                    # one multiply for both heads: the mask AP broadcasts
                    # over the head dim with a 0-stride
                    mqb = bass.AP(tensor=mq.tensor, offset=mq.offset,
                                  ap=[mq.ap[0], [0, 2]] + list(mq.ap[1:]))
                    nc.vector.tensor_tensor(out=pt, in0=pt, in1=mqb,
                                            op=OP.mult)
                return pt

            def back(gb, pt):
                for gi in range(2):
                    g = gb + gi
                    nc.tensor.matmul(cx_e, lhsT=Vaug[:, g, 2 * hp, :],
                                     rhs=pt[:, 0, gi, :],
                                     start=(g == 0), stop=(g == ext - 1))
                    nc.tensor.matmul(cx_o, lhsT=Vaug[:, g, 2 * hp + 1, :],
                                     rhs=pt[:, 1, gi, :],
                                     start=(g == 0), stop=(g == ext - 1))
                if gb == 2 and pending:
                    pending.pop()()
                if fill and (qb == 0 or gb % 4 == 2):
                    fill.pop(0)()

            # one-pair software pipeline: pair t+1's score matmuls issue
            # BEFORE pair t's ctx matmuls/fillers, so the exp chain on ACT
            # (the qb1 bottleneck) never starves behind PE queue order --
            # S(t,a) S(t,b) C(t-1) takes ~1.7us, just past exp(t,a)'s 1.57us
            prev = None
            for gb in range(0, ext, 2):
                pt = front(gb)
                if prev is not None:
                    back(*prev)
                prev = (gb, pt)
            back(*prev)
            # stage ctx to SBUF immediately: frees the PSUM bank within one
            # DVE copy so the cx pool gets away with a single buffer
            cxs_e = pools['nm_sb'].tile([DK + 1, QBS], F32, tag="cxs_e")
            nc.vector.tensor_copy(out=cxs_e, in_=cx_e)
            cxs_o = pools['nm_sb'].tile([DK + 1, QBS], F32, tag="cxs_o")
            nc.vector.tensor_copy(out=cxs_o, in_=cx_o)
            den2 = pools['nm_sb'].tile([DK + 1, QBS], F32, tag="den2")
            nc.vector.memset(den2, 1.0)
            nc.vector.tensor_copy(out=den2[0:1, :], in_=cx_e[DK : DK + 1, :])
            nc.vector.tensor_copy(out=den2[DK : DK + 1, :], in_=cx_o[DK : DK + 1, :])
            pending.append(make_norm(cxs_e, cxs_o, den2, hp, qs))

        # ---------- schedule: projections + qb0 attention ----------
        for hp in range(HP):
            q_unit(hp, 0)
        kv_slab(0, on_act=False)
        kv_slab(1, on_act=False)
        for hp in range(HP):
            q_unit(hp, 1)
        q_in.release()
        xt01p.release()
        pools['at_sb'] = tc.alloc_tile_pool(name="at_sb", bufs=3)
        pools['nm_sb'] = tc.alloc_tile_pool(name="nm_sb", bufs=2)
        mT0p = tc.alloc_tile_pool(name="mT0p", bufs=1)
        pools['mT0'] = mT0p.tile([P, 8, QBS], BF16, name="mT0", tag="mT0")
        nc.gpsimd.dma_start(out=pools['mT0'], in_=mr[:, 0:8, 0:QBS])
        # on_act=False: these run woven into qb0 attention where ACT is the
        # exp-bound critical chain -- their epilogues go to DVE instead
        kv_fill = [(lambda hp=hp, sb=sb: k_unit(hp, sb, on_act=False))
                   for sb in (2, 3) for hp in range(HP)] + \
                  [(lambda kb=kb, nh=nh: v_unit(kb, nh, on_act=False))
                   for kb in range(8, KB) for nh in range(NH)]
        for hp in range(HP):
            attn_iter(hp, 0, kv_fill)
        for fn in kv_fill:
            fn()
        kv_fill.clear()
        kv_in.release()
        proj_ps.release()
        mT0p.release()

        # ---------- qb1 attention with qb0 epilogue woven in ----------
        mid_ow = tc.alloc_tile_pool(name="mid_ow", bufs=1, side="right")
        xres_sb = mid_ow.tile([P, QC, D], F32)
        nc.gpsimd.dma_start(out=xres_sb,
                            in_=xres.rearrange("(c p) n -> p c n", p=P))
        wo_sb = mid_ow.tile([P, DC, D], BF16)
        nc.gpsimd.dma_start(out=wo_sb, in_=wo.rearrange("(c p) n -> p c n", p=P))
        op_ps = tc.alloc_tile_pool(name="op_ps", bufs=1, space="PSUM", side="right")
        tp_ps = tc.alloc_tile_pool(name="tp_ps", bufs=1, space="PSUM", side="right")

        hpre_map = {}

        def op_half(qc, nh):
            def go():
                if qc not in hpre_map:
                    hpre_map[qc] = ln_wk.tile([P, D], F32, tag="hpre",
                                              bufs=3, name=f"hpre_{qc}")
                hpre = hpre_map[qc]
                ps = op_ps.tile([P, NHW], F32, tag="op")
                for c in range(DC):
                    nc.tensor.matmul(
                        ps, lhsT=ctxT[:, c, qc * P : (qc + 1) * P],
                        rhs=wo_sb[:, c, nh * NHW : (nh + 1) * NHW],
                        start=(c == 0), stop=(c == DC - 1),
                    )
                nc.vector.scalar_tensor_tensor(
                    out=hpre[:, nh * NHW : (nh + 1) * NHW], in0=ps,
                    scalar=1.0, in1=xres_sb[:, qc, nh * NHW : (nh + 1) * NHW],
                    op0=OP.mult, op1=OP.add,
                )
            return go

        def ln_unit(qc):
            def go():
                layer_norm(ln_wk, hpre_map.pop(qc), g1b, be1b, h_sb[:, qc, :])
            return go

        def transp_half(qc, lo):
            def go():
                for c in range(lo, lo + DC // 2):
                    tp = tp_ps.tile([P, P], BF16, tag="tp")
                    nc.tensor.transpose(tp, h_sb[:, qc, c * P : (c + 1) * P],
                                        ident)
                    nc.scalar.activation(out=hT[:, c, qc * P : (qc + 1) * P],
                                         in_=tp, func=AF.Copy)
            return go

        def outproj_unit(qc):
            def go():
                op_half(qc, 0)()
                op_half(qc, 1)()
                ln_unit(qc)()
            return go

        def transp_unit(qc):
            def go():
                for c in range(DC):
                    tp = tp_ps.tile([P, P], BF16, tag="tp")
                    nc.tensor.transpose(tp, h_sb[:, qc, c * P : (c + 1) * P],
                                        ident)
                    nc.scalar.activation(out=hT[:, c, qc * P : (qc + 1) * P],
                                         in_=tp, func=AF.Copy)
            return go

        fillers = []
        for qc in range(4):
            fillers += [op_half(qc, 0), op_half(qc, 1), ln_unit(qc)]
        for qc in range(4):
            fillers += [transp_half(qc, 0), transp_half(qc, DC // 2)]
        for hp in range(HP):
            attn_iter(hp, 1, fillers)
        for fn in pending:
            fn()
        pending.clear()
        for fn in fillers:
            fn()
        pools['nm_sb'].release()
        pools['at_sb'].release()

    attn_in.release()

    # prefetch the 9.4MB w1/w2 load so it overlaps the qc4-7 epilogue
    # instead of stalling the FFN phase start
    w12_in = tc.alloc_tile_pool(name="w12_in", bufs=1)
    w1_sb = w12_in.tile([P, DC, DF], BF16)
    nc.sync.dma_start(out=w1_sb, in_=w1.rearrange("(c p) n -> p c n", p=P))
    w2_sb = w12_in.tile([P, DFC, D], F8E4)
    nc.scalar.dma_start(out=w2_sb, in_=w2.rearrange("(c p) n -> p c n", p=P))

    # ====== FFN: f1^T = gelu(w1^T h^T + b1); out = LN2(f1g^T w2 + h) ====
    with tc.tile_pool(name="f1_ps", bufs=3, space="PSUM") as f1_ps, \
         tc.tile_pool(name="f2_ps", bufs=3, space="PSUM") as f2_ps, \
         tc.tile_pool(name="f1g_sb", bufs=2) as f1g_sb, \
         tc.tile_pool(name="out_sb", bufs=3) as out_sb:

        def ffn1(qb, fill=None):
            qs = slice(qb * QBS, (qb + 1) * QBS)
            # fp8: FC2 runs as DoubleRow (2 k-tiles/pass, ~1.4x); gelu
            # outputs |g|<~8 and w2~0.02 are far inside e4m3 range, and the
            # FC2-only quantization error (~1.4e-2) fits the 2e-2 tolerance
            f1g = f1g_sb.tile([P, DFC, QBS], F8E4, tag="f1g",
                              name=f"f1g_{qb}")
            for f in range(DFC):
                ps = f1_ps.tile([P, QBS], F32, tag="f1")
                for c in range(DC):
                    nc.tensor.matmul(
                        ps, lhsT=w1_sb[:, c, f * P : (f + 1) * P],
                        rhs=hT[:, c, qs], start=(c == 0), stop=(c == DC - 1),
                    )
                nc.scalar.activation(out=f1g[:, f, :], in_=ps, func=AF.Gelu,
                                     bias=b1p[:, f : f + 1])
                if fill:
                    fill.pop(0)()
            return f1g

        def ffn2(qb, f1g):
            for sq in range(QBS // P):
                qc = qb * (QBS // P) + sq
                ot = out_sb.tile([P, D], F32, tag="ot")
                for nh in range(NH):
                    ps = f2_ps.tile([P, NHW], F32, tag="f2")
                    for fp in range(DFC // 2):
                        nc.tensor.matmul(
                            ps,
                            lhsT=f1g[:, 2 * fp : 2 * fp + 2,
                                     sq * P : (sq + 1) * P],
                            rhs=w2_sb[:, 2 * fp : 2 * fp + 2,
                                      nh * NHW : (nh + 1) * NHW],
                            start=(fp == 0), stop=(fp == DFC // 2 - 1),
                            perf_mode=mybir.MatmulPerfMode.DoubleRow,
                        )
                    nc.vector.scalar_tensor_tensor(
                        out=ot[:, nh * NHW : (nh + 1) * NHW], in0=ps,
                        scalar=1.0,
                        in1=h_sb[:, qc, nh * NHW : (nh + 1) * NHW],
                        op0=OP.mult, op1=OP.add,
                    )
                nc.vector.tensor_tensor(out=ot, in0=ot, in1=b2b, op=OP.add)
                layer_norm(ln_wk, ot, g2b, be2b, ot)
                nc.sync.dma_start(out=out[qc * P : (qc + 1) * P, :], in_=ot)

        # ---- qc4-7 out-projection epilogue woven INTO FFN1(qb0): FFN1 qb0
        # only needs hT qc0-3 (built during the qb1 fillers). Each op_half
        # rides between two dense f-units so its bufs=1 PSUM ring drains
        # behind PE work instead of serializing against the DVE adds
        epi = []
        for qc in range(4, QC):
            epi += [op_half(qc, 0), op_half(qc, 1), ln_unit(qc)]
        f1g0 = ffn1(0, epi)
        for qc in range(4, QC):
            transp_unit(qc)()
        ffn2(0, f1g0)
        ffn2(1, ffn1(1))

    tp_ps.release()
    op_ps.release()
    mid_ow.release()
    mid_ctx.release()

    w12_in.release()
    ln_wk.release()
    ffn.release()


def build_program():
    nc = bacc.Bacc("TRN2", target_bir_lowering=False, debug=False,
                   enable_asserts=False, num_devices=N_CORES)
    io = {}

    def din(name, shape, dt):
        io[name] = nc.dram_tensor(name, list(shape), dt, kind="ExternalInput").ap()

    din("xT", (D, S), BF16)
    din("xqT", (D, SQ), BF16)
    din("xres", (SQ, D), F32)
    din("maskT", (S, SQ), BF16)
    din("wq", (D, D), BF16)
    din("wk", (D, D), BF16)
    din("wv", (D, D), BF16)
    din("wo", (D, D), BF16)
    din("w1", (D, DF), BF16)
    din("w2", (DF, D), F8E4)
    din("bqp", (P, HP), F32)
    din("bkp", (P, HP), F32)
    din("b1p", (P, DFC), F32)
    for n in ["g1r", "be1r", "g2r", "be2r", "b2r"]:
        din(n, (1, D), F32)
    io["out"] = nc.dram_tensor("out", [SQ, D], F32, kind="ExternalOutput").ap()

    with tile.TileContext(nc) as tc:
        with ExitStack() as ctx:
            emit(ctx, tc, io)
    nc.compile()
    return nc


_NC = None


def _get_program():
    global _NC
    if _NC is None:
        _NC = build_program()
    return _NC


def _qrows(half):
    if half == 0:
        return np.concatenate([np.arange(0, 512), np.arange(1536, 2048)])
    return np.arange(512, 1536)


def shard_inputs(inputs):
    x = np.asarray(inputs["x"], np.float32)
    mask = np.asarray(inputs["mask"], bool)
    w = {k: np.asarray(inputs[k], np.float32) for k in
         ["wq", "bq", "wk", "bk", "wv", "bv", "wo", "bo", "g1", "be1",
          "w1", "b1", "w2", "b2", "g2", "be2"]}

    base = dict(
        wq=np.ascontiguousarray(w["wq"].astype(BF)),
        wk=np.ascontiguousarray(w["wk"].astype(BF)),
        wv=np.ascontiguousarray(w["wv"].astype(BF)),
        wo=np.ascontiguousarray(w["wo"].astype(BF)),
        w1=np.ascontiguousarray(w["w1"].astype(BF)),
        w2=np.ascontiguousarray(w["w2"].astype(ml_dtypes.float8_e4m3)),
        bqp=np.ascontiguousarray(w["bq"].reshape(HP, P).T),
        bkp=np.ascontiguousarray(w["bk"].reshape(HP, P).T),
        b1p=np.ascontiguousarray(w["b1"].reshape(DFC, P).T),
        g1r=np.ascontiguousarray(w["g1"].reshape(1, D)),
        be1r=np.ascontiguousarray(w["be1"].reshape(1, D)),
        g2r=np.ascontiguousarray(w["g2"].reshape(1, D)),
        be2r=np.ascontiguousarray(w["be2"].reshape(1, D)),
        b2r=np.ascontiguousarray(w["b2"].reshape(1, D)),
    )
    # bv and bo fold into the residual: ctx@wo + bo + x with v-bias bv adds
    # a constant row bv@wo (softmax rows sum to 1)
    res_const = (w["bo"] + w["bv"] @ w["wo"]).astype(np.float32)

    in_maps = []
    for c in range(N_CORES):
        b, half = divmod(c, 2)
        qr = _qrows(half)
        xb = x[b]
        xq = xb[qr]
        m = dict(base)
        m["xT"] = np.ascontiguousarray(xb.T.astype(BF))
        m["xqT"] = np.ascontiguousarray(xq.T.astype(BF))
        m["xres"] = np.ascontiguousarray(xq + res_const[None, :])
        m["maskT"] = np.ascontiguousarray((~mask[b][qr]).T.astype(BF))
        in_maps.append(m)
    return in_maps


def gather_outputs(results):
    y = np.empty((B, S, D), np.float32)
    for c in range(N_CORES):
        b, half = divmod(c, 2)
        y[b, _qrows(half)] = results[c]["out"]
    return y


def kernel(**inputs):
    nc = _get_program()
    in_maps = shard_inputs(inputs)
    res = run_bass_kernel_spmd(nc, in_maps, list(range(N_CORES)))
    return gather_outputs(res.results)


if __name__ == "__main__":
    build_program()
    print("program built ok")

